# revision 24
# baseline (speedup 1.0000x reference)
"""CascadeRCNN head (3-stage cascade + test-time ensemble) on 8 Trainium2 NeuronCores.

Data-parallel over rois: 1000 rois sharded 8 x 125; FPN feature maps and head
weights replicated. Everything (ROIAlign gather+bilinear, FC GEMMs, softmax,
delta2bbox) runs on-device; host only shards/concats.

Precision plan (tolerance 2e-2; measured headroom ~45x at fp32/bf16 baseline):
 - FPN features: bf16 (halves the gather traffic).
 - fc1/fc2 weights: fp8 e3m4, scaled by 128/64 (power of two, exact descale
   folded into the post-GEMM activation). PSUM accumulation stays fp32.
 - cls/reg weights: bf16. Activations x/h1T/h2T: bf16 lhsT, fp32 psum.
This cuts per-core HBM traffic from ~265MB to ~109MB; fc1 weight streams are
loaded in 7-k-tile (896KB) chunks to stay transfer-bound, not issue-bound.
"""

import numpy as np
from contextlib import ExitStack

import concourse.bass as bass
import concourse.tile as tile
from concourse import bacc, mybir
from concourse.masks import make_identity

F32 = mybir.dt.float32
BF16 = mybir.dt.bfloat16
FP8 = mybir.dt.float8e3
FP8E4 = mybir.dt.float8e4
I32 = mybir.dt.int32
Alu = mybir.AluOpType
Act = mybir.ActivationFunctionType

N_CORES = 8
R = 125              # rois per core
POOL = 7
C = 256
K1 = 12544           # 7*7*256
KT1 = 98             # fc1 k-tiles
CH1 = 7              # fc1 k-tiles per weight DMA
HID = 1024
KT2 = 8
NCLS = 81
IMG = 1024.0
S1 = 128.0           # fc1 weight fp8 scale (power of two)
S2 = 64.0            # fc2 weight fp8 scale

# concatenated feature table: P2 (256x256), P3 (128x128), P4 (64x64), P5 (32x32)
FEAT_ROWS = 256 * 256 + 128 * 128 + 64 * 64 + 32 * 32  # 87040

N_STAGES = 3
DR_ON = False   # DoubleRow fp8e4 for the three probs-path fc1 GEMMs


def _roi_prep(nc, pools, rois_t, grid_t):
    """From rois [R,4] compute gather indices and bilinear weights.

    Returns (idx_i32 [R, 98], wx_eff [R,7], wy_eff [R,7]).
    idx free layout: (jy, jx, yn) -> col jy*14 + jx*2 + yn. Each index is a row
    of the feats table; a gather of 512 elems covers pixel columns (bx, bx+1).
    """
    prep = pools["prep"]
    v = nc.vector

    def pt(cols, dtype=F32, tag=None):
        return prep.tile([R, cols], dtype, tag=tag, name=tag)

    y1 = rois_t[:, 0:1]
    x1 = rois_t[:, 1:2]
    y2 = rois_t[:, 2:3]
    x2 = rois_t[:, 3:4]

    hh = pt(1, tag="hh"); v.tensor_tensor(hh[:], y2, y1, op=Alu.subtract)
    ww = pt(1, tag="ww"); v.tensor_tensor(ww[:], x2, x1, op=Alu.subtract)
    hw = pt(1, tag="hw"); v.tensor_tensor(hw[:], hh[:], ww[:], op=Alu.mult)
    v.tensor_scalar(hw[:], hw[:], 1e-6, None, op0=Alu.max)

    # level selection: lvl = 2 + (hw>=112^2) + (hw>=224^2) + (hw>=448^2)
    g2 = pt(1, tag="g2"); v.tensor_scalar(g2[:], hw[:], 12544.0, None, op0=Alu.is_ge)
    g3 = pt(1, tag="g3"); v.tensor_scalar(g3[:], hw[:], 50176.0, None, op0=Alu.is_ge)
    g4 = pt(1, tag="g4"); v.tensor_scalar(g4[:], hw[:], 200704.0, None, op0=Alu.is_ge)

    # inv_stride = 0.25 - 0.125*g2 - 0.0625*g3 - 0.03125*g4  (exact)
    invs = pt(1, tag="invs")
    v.tensor_scalar(invs[:], g2[:], -0.125, 0.25, op0=Alu.mult, op1=Alu.add)
    t0 = pt(1, tag="t0")
    v.tensor_scalar(t0[:], g3[:], -0.0625, None, op0=Alu.mult)
    v.tensor_tensor(invs[:], invs[:], t0[:], op=Alu.add)
    v.tensor_scalar(t0[:], g4[:], -0.03125, None, op0=Alu.mult)
    v.tensor_tensor(invs[:], invs[:], t0[:], op=Alu.add)

    # feature side S = 1024 * inv_stride in {256,128,64,32}; level base offset
    S = pt(1, tag="S"); v.tensor_scalar(S[:], invs[:], 1024.0, None, op0=Alu.mult)
    base = pt(1, tag="base")
    v.tensor_scalar(base[:], g2[:], 65536.0, None, op0=Alu.mult)
    v.tensor_scalar(t0[:], g3[:], 16384.0, None, op0=Alu.mult)
    v.tensor_tensor(base[:], base[:], t0[:], op=Alu.add)
    v.tensor_scalar(t0[:], g4[:], 4096.0, None, op0=Alu.mult)
    v.tensor_tensor(base[:], base[:], t0[:], op=Alu.add)
    Sm1 = pt(1, tag="Sm1"); v.tensor_scalar(Sm1[:], S[:], -1.0, None, op0=Alu.add)
    Sm2 = pt(1, tag="Sm2"); v.tensor_scalar(Sm2[:], S[:], -2.0, None, op0=Alu.add)

    # scaled roi coords (exact: multiply by power of two)
    sy1 = pt(1, tag="sy1"); v.tensor_tensor(sy1[:], y1, invs[:], op=Alu.mult)
    sx1 = pt(1, tag="sx1"); v.tensor_tensor(sx1[:], x1, invs[:], op=Alu.mult)
    sy2 = pt(1, tag="sy2"); v.tensor_tensor(sy2[:], y2, invs[:], op=Alu.mult)
    sx2 = pt(1, tag="sx2"); v.tensor_tensor(sx2[:], x2, invs[:], op=Alu.mult)
    dy = pt(1, tag="dy"); v.tensor_tensor(dy[:], sy2[:], sy1[:], op=Alu.subtract)
    dx = pt(1, tag="dx"); v.tensor_tensor(dx[:], sx2[:], sx1[:], op=Alu.subtract)

    def axis_prep(scoord, dcoord, suffix, clamp):
        # ys = grid*d + s  (matches ref rounding: mult then add)
        ys = pt(POOL, tag="ys" + suffix)
        v.tensor_scalar(ys[:], grid_t[0:R, :], dcoord[:], scoord[:],
                        op0=Alu.mult, op1=Alu.add)
        # robust floor (works under truncation or round-to-nearest casts)
        yi = pt(POOL, I32, tag="yi" + suffix)
        v.tensor_copy(yi[:], ys[:])
        yf = pt(POOL, tag="yf" + suffix)
        v.tensor_copy(yf[:], yi[:])
        gt = pt(POOL, tag="gt" + suffix)
        v.tensor_tensor(gt[:], yf[:], ys[:], op=Alu.is_gt)
        y0f = pt(POOL, tag="y0f" + suffix)
        v.tensor_tensor(y0f[:], yf[:], gt[:], op=Alu.subtract)
        # clip to [0, S-1] (ys >= 0 so lower clip is a no-op)
        y0c = pt(POOL, tag="y0c" + suffix)
        v.tensor_scalar(y0c[:], y0f[:], Sm1[:], None, op0=Alu.min)
        # index clamp: x needs bx+1 <= S-1 (contiguous pair read); y is baked
        # into the paired table, so its row index is y0c itself.
        by = pt(POOL, tag="by" + suffix)
        v.tensor_scalar(by[:], y0c[:], clamp[:], None, op0=Alu.min)
        # weight: clip(ys - y0c, 0, 1); force 1.0 at the right edge
        wy = pt(POOL, tag="wy" + suffix)
        v.tensor_tensor(wy[:], ys[:], y0c[:], op=Alu.subtract)
        v.tensor_scalar(wy[:], wy[:], 0.0, 1.0, op0=Alu.max, op1=Alu.min)
        fl = pt(POOL, tag="fl" + suffix)
        v.tensor_scalar(fl[:], y0f[:], Sm1[:], None, op0=Alu.is_ge)
        v.tensor_tensor(wy[:], wy[:], fl[:], op=Alu.max)
        return by, wy

    by, wy_eff = axis_prep(sy1, dy, "Y", Sm1)
    bx, wx_eff = axis_prep(sx1, dx, "X", Sm2)

    # rowA = base + by*S  [R,7]
    rowA = pt(POOL, tag="rowA")
    v.tensor_scalar(rowA[:], by[:], S[:], base[:], op0=Alu.mult, op1=Alu.add)

    # idx layout [R, 7jy, 7jx]; one paired-table row index per bin. The
    # paired table row (y, x) holds [feat(y, x), feat(min(y+1, S-1), x)], so a
    # 1024-elem read from row (base + y*S + x) covers all 4 bilinear
    # neighbors: [f(y,x0), f(y+1,x0), f(y,x0+1), f(y+1,x0+1)].
    idxf = pools["idx"].tile([R, POOL * POOL], F32, tag="idxf", name="idxf")
    v3 = idxf[:].rearrange("p (a b) -> p a b", a=POOL)
    v.tensor_tensor(v3[:, :, :],
                    rowA[:, :, None].to_broadcast([R, POOL, POOL]),
                    bx[:, None, :].to_broadcast([R, POOL, POOL]),
                    op=Alu.add)
    idx_i32 = pools["idx"].tile([R, POOL * POOL], I32, tag="idxi", name="idxi")
    v.tensor_copy(idx_i32[:], idxf[:])
    return idx_i32, wx_eff, wy_eff


def _gather_interp_transpose(nc, pools, feats_ap, idx_i32, wx_eff, wy_eff,
                             ident_b, xt_tiles):
    """Fill xt_tiles (7 tiles of [128, 14*128]; K-block k at cols
    (k%14)*128, 125 used) with pooled features transposed (x^T for fc1).
    The psum->SBUF copy converts to each xt tile's dtype (bf16 or fp8e4)."""
    v = nc.vector
    # Per-stage x-interp diagonal weight tiles: Wd[jx][0] = diag(1-wx[:,jx]),
    # Wd[jx][1] = diag(wx[:,jx]). The x-interp then folds into the PE
    # "transposes" as psum-accumulated diagonal matmuls:
    #   xt[c, roi] = (1-wx)(roi)*T0[roi, c] + wx(roi)*T1[roi, c]
    wxm = pools["prep"].tile([R, POOL], F32, tag="wxm", name="wxm")
    v.tensor_scalar(wxm[:], wx_eff[:], -1.0, 1.0, op0=Alu.mult, op1=Alu.add)
    wxd = pools["wxd"].tile([R, POOL * 2 * R], BF16, tag="wxd", name="wxd")
    wdv = wxd[:].rearrange("p (b x q) -> p b x q", b=POOL, x=2)
    for jx in range(POOL):
        v.tensor_scalar(wdv[:, jx, 0, :], ident_b[0:R, 0:R],
                        wxm[:, jx:jx + 1], None, op0=Alu.mult)
        v.tensor_scalar(wdv[:, jx, 1, :], ident_b[0:R, 0:R],
                        wx_eff[:, jx:jx + 1], None, op0=Alu.mult)
    for jy in range(POOL):
        G = pools["gath"].tile([R, 7 * 1024], BF16, tag="G", name="G")
        # HW indirect DMA consumes exactly one index per output partition,
        # so issue one gather per jx bin; the paired table packs all 4
        # bilinear neighbors into one 1024-elem contiguous read.
        for j in range(POOL):
            nc.gpsimd.indirect_dma_start(
                out=G[:, j * 1024:(j + 1) * 1024],
                out_offset=None,
                in_=feats_ap[:],
                in_offset=bass.IndirectOffsetOnAxis(
                    ap=idx_i32[:, jy * POOL + j:jy * POOL + j + 1], axis=0),
            )
        # layout [R, jx, xn, yn, c] (bf16 keeps the DVE 2x/4x fast modes)
        gv = G[:].rearrange("p (b x y e) -> p b x y e", b=POOL, x=2, y=2)
        # y-interp in place: G1 <- G1-G0; G1 <- wy*G1; G0 <- G0+G1 (= T)
        v.tensor_tensor(gv[:, :, :, 1, :], gv[:, :, :, 1, :],
                        gv[:, :, :, 0, :], op=Alu.subtract)
        nc.scalar.mul(gv[:, :, :, 1, :], gv[:, :, :, 1, :],
                      wy_eff[:, jy:jy + 1])
        v.tensor_tensor(gv[:, :, :, 0, :], gv[:, :, :, 0, :],
                        gv[:, :, :, 1, :], op=Alu.add)
        # x-interp + transpose fused on the PE: per (jx, c-half) accumulate
        # the two xn diagonal matmuls into one psum slot; two slots share a
        # PSUM bank so each psum->SBUF copy moves a pair.
        xt = xt_tiles[jy]
        xv = xt[:].rearrange("p (t c) -> p t c", c=128)
        for jx in range(POOL):
            ps = pools["pt"].tile([128, 2, 128], F32, space="PSUM",
                                  tag="ptrf", name="ptrf")
            for u in range(2):
                for xn in range(2):
                    nc.tensor.matmul(
                        ps[:, u, 0:R],
                        lhsT=gv[:, jx, xn, 0, u * 128:(u + 1) * 128],
                        rhs=wdv[:, jx, xn, :],
                        start=(xn == 0), stop=(xn == 1))
            nc.scalar.copy(xv[:, 2 * jx:2 * jx + 2, 0:R], ps[:, :, 0:R])


def _fc(nc, pools, xt_slices, KT, w_tiled, b2, head, n_out, relu, ones, w_tag,
        wdt, chunk, inv_scale=1.0, h_dt=F32, dr=False):
    """h[R, n_out] = act((x @ W)*inv_scale + b*inv_scale).

    W pre-tiled on host as [3, KT//chunk, 128, chunk*n_out]; b pre-scaled by
    1/inv_scale so the single post-GEMM activation descale is exact.
    xt_slices(k) -> lhsT AP [128, R]. With dr=True both operands are fp8e4
    and k-tiles are consumed in DoubleRow pairs (xt_slices(k) must then
    return a [128, 2, R] pair AP). PSUM matmul outputs must stay within one
    2KB bank, hence the 512-column psum split."""
    offs = list(range(0, n_out, 512))
    sizes = [min(512, n_out - o) for o in offs]
    psums = [pools["pfc"].tile([R, 512], F32, space="PSUM", tag=f"ps{j}",
                               name=f"ps{j}")
             for j in range(len(offs))]
    dma_eng = nc.sync if w_tag in ("w1", "w4") else nc.scalar
    n_ch = KT // chunk
    for c in range(n_ch):
        wt = pools[w_tag].tile([128, chunk * n_out], wdt, tag=w_tag, name=w_tag)
        dma_eng.dma_start(wt[:], w_tiled[head, c])
        if dr:
            wv = wt[:].rearrange("p (a n) -> p a n", a=chunk)
            for u in range(chunk // 2):
                k = c * chunk + 2 * u
                for jj, (o, sz) in enumerate(zip(offs, sizes)):
                    nc.tensor.matmul(
                        psums[jj][:, 0:sz], lhsT=xt_slices(k),
                        rhs=wv[:, 2 * u:2 * u + 2, o:o + sz],
                        start=(k == 0), stop=False,
                        perf_mode=mybir.MatmulPerfMode.DoubleRow)
            continue
        for j in range(chunk):
            k = c * chunk + j
            for jj, (o, sz) in enumerate(zip(offs, sizes)):
                nc.tensor.matmul(psums[jj][:, 0:sz], lhsT=xt_slices(k),
                                 rhs=wt[:, j * n_out + o:j * n_out + o + sz],
                                 start=(k == 0), stop=False)
    bt = pools["bias"].tile([1, n_out], F32, tag="bias", name="bias")
    nc.scalar.dma_start(bt[:], b2[head][None, :])
    for jj, (o, sz) in enumerate(zip(offs, sizes)):
        nc.tensor.matmul(psums[jj][:, 0:sz], lhsT=ones[0:1, 0:R],
                         rhs=bt[0:1, o:o + sz], start=False, stop=True)
    h = pools["h"].tile([R, n_out], h_dt, tag=f"h{n_out}", name=f"h{n_out}")
    for jj, (o, sz) in enumerate(zip(offs, sizes)):
        if relu:
            nc.scalar.activation(h[:, o:o + sz], psums[jj][:, 0:sz], Act.Relu,
                                 scale=inv_scale)
        elif inv_scale != 1.0:
            nc.scalar.activation(h[:, o:o + sz], psums[jj][:, 0:sz], Act.Copy,
                                 scale=inv_scale)
        else:
            nc.scalar.copy(h[:, o:o + sz], psums[jj][:, 0:sz])
    return h


def _transpose_h(nc, pools, h, ident):
    """h [R, 1024] fp32 -> hT [128, 1024] bf16 (8 blocks, 125 cols used)."""
    hT = pools["ht"].tile([128, HID], BF16, tag="hT", name="hT")
    hv = hT[:].rearrange("p (t c) -> p t c", c=128)
    for t in range(0, HID // 128, 2):
        ps = pools["pt"].tile([128, 2, 128], F32, space="PSUM", tag="ptrf",
                              name="ptrf")
        for u in range(2):
            nc.tensor.transpose(out=ps[:, u, 0:R],
                                in_=h[:, (t + u) * 128:(t + u + 1) * 128],
                                identity=ident[0:R, 0:R])
        nc.scalar.copy(hv[:, t:t + 2, 0:R], ps[:, :, 0:R])
    return hT


def _softmax(nc, pools, logits):
    v = nc.vector
    rmax = pools["prep"].tile([R, 1], F32, tag="rmax", name="rmax")
    v.tensor_reduce(rmax[:], logits[:], axis=mybir.AxisListType.X, op=Alu.max)
    nmax = pools["prep"].tile([R, 1], F32, tag="nmax", name="nmax")
    v.tensor_scalar(nmax[:], rmax[:], -1.0, None, op0=Alu.mult)
    e = pools["h"].tile([R, NCLS], F32, tag="smx", name="smx")
    nc.scalar.activation(e[:], logits[:], Act.Exp, bias=nmax[:], scale=1.0)
    ssum = pools["prep"].tile([R, 1], F32, tag="ssum", name="ssum")
    v.tensor_reduce(ssum[:], e[:], axis=mybir.AxisListType.X, op=Alu.add)
    rsum = pools["prep"].tile([R, 1], F32, tag="rsum", name="rsum")
    v.reciprocal(rsum[:], ssum[:])
    v.tensor_scalar(e[:], e[:], rsum[:], None, op0=Alu.mult)
    return e


def _delta2bbox(nc, pools, rois_t, deltas, stds_t, rois_pool):
    """rois_next = delta2bbox(rois_t, deltas) following the reference op order."""
    v = nc.vector
    prep = pools["prep"]

    def pt(tag):
        return prep.tile([R, 1], F32, tag=tag, name=tag)

    d = prep.tile([R, 4], F32, tag="dsc", name="dsc")
    v.tensor_tensor(d[:], deltas[:], stds_t[0:R, :], op=Alu.mult)
    y1 = rois_t[:, 0:1]; x1 = rois_t[:, 1:2]; y2 = rois_t[:, 2:3]; x2 = rois_t[:, 3:4]
    hh = pt("b_h"); v.tensor_tensor(hh[:], y2, y1, op=Alu.subtract)
    ww = pt("b_w"); v.tensor_tensor(ww[:], x2, x1, op=Alu.subtract)
    # cy = (y1 + 0.5*h) + d0*h ; cx likewise
    hh2 = pt("b_h2"); v.tensor_scalar(hh2[:], hh[:], 0.5, None, op0=Alu.mult)
    cy = pt("b_cy"); v.tensor_tensor(cy[:], y1, hh2[:], op=Alu.add)
    t = pt("b_t"); v.tensor_tensor(t[:], d[:, 0:1], hh[:], op=Alu.mult)
    v.tensor_tensor(cy[:], cy[:], t[:], op=Alu.add)
    ww2 = pt("b_w2"); v.tensor_scalar(ww2[:], ww[:], 0.5, None, op0=Alu.mult)
    cx = pt("b_cx"); v.tensor_tensor(cx[:], x1, ww2[:], op=Alu.add)
    v.tensor_tensor(t[:], d[:, 1:2], ww[:], op=Alu.mult)
    v.tensor_tensor(cx[:], cx[:], t[:], op=Alu.add)
    # h' = h*exp(d2), w' = w*exp(d3)
    eh = pt("b_eh"); nc.scalar.activation(eh[:], d[:, 2:3], Act.Exp)
    ew = pt("b_ew"); nc.scalar.activation(ew[:], d[:, 3:4], Act.Exp)
    v.tensor_tensor(hh[:], hh[:], eh[:], op=Alu.mult)
    v.tensor_tensor(ww[:], ww[:], ew[:], op=Alu.mult)
    v.tensor_scalar(hh2[:], hh[:], 0.5, None, op0=Alu.mult)
    v.tensor_scalar(ww2[:], ww[:], 0.5, None, op0=Alu.mult)
    rn = rois_pool.tile([R, 4], F32, tag="rois", name="rois")
    v.tensor_tensor(rn[:, 0:1], cy[:], hh2[:], op=Alu.subtract)
    v.tensor_tensor(rn[:, 1:2], cx[:], ww2[:], op=Alu.subtract)
    v.tensor_tensor(rn[:, 2:3], cy[:], hh2[:], op=Alu.add)
    v.tensor_tensor(rn[:, 3:4], cx[:], ww2[:], op=Alu.add)
    for j in range(4):
        v.tensor_scalar(rn[:, j:j + 1], rn[:, j:j + 1], 0.0, IMG,
                        op0=Alu.max, op1=Alu.min)
    return rn


def build_kernel(ctx: ExitStack, tc: "tile.TileContext", aps: dict):
    nc = tc.nc
    pools = {}
    for name, bufs, space in [
        ("const", 1, "SBUF"), ("rois", 2, "SBUF"), ("prep", 2, "SBUF"),
        ("idx", 2, "SBUF"), ("gath", 3, "SBUF"), ("wxd", 2, "SBUF"),
        ("xt", 1, "SBUF"), ("w1", 3, "SBUF"), ("w4", 2, "SBUF"),
        ("w2", 2, "SBUF"),
        ("wsm", 2, "SBUF"), ("bias", 2, "SBUF"), ("h", 2, "SBUF"),
        ("ht", 2, "SBUF"), ("acc", 1, "SBUF"),
        ("pt", 2, "PSUM"), ("pfc", 2, "PSUM"),
    ]:
        pools[name] = ctx.enter_context(tc.tile_pool(name=name, bufs=bufs,
                                                     space=space))

    ident = pools["const"].tile([128, 128], F32, tag="ident", name="ident")
    make_identity(nc, ident[:])
    ident_b = pools["const"].tile([128, 128], BF16, tag="identb", name="identb")
    nc.vector.tensor_copy(ident_b[:], ident[:])
    ones = pools["const"].tile([1, 128], F32, tag="ones", name="ones")
    nc.vector.memset(ones[:], 1.0)
    grid_t = pools["const"].tile([128, POOL], F32, tag="grid", name="grid")
    nc.sync.dma_start(grid_t[:], aps["grid_c"][:])
    stds_t = pools["const"].tile([128, 4], F32, tag="stds", name="stds")
    nc.sync.dma_start(stds_t[:], aps["stds_c"][:])

    rois_t = pools["rois"].tile([R, 4], F32, tag="rois", name="rois")
    nc.sync.dma_start(rois_t[:], aps["rois"][:])

    def head_app(i, xt_tiles, want, dr=False):
        if dr:
            def xt_slice(k):
                xv = xt_tiles[k // 14][:].rearrange("p (t c) -> p t c", c=128)
                return xv[:, k % 14:k % 14 + 2, 0:R]

            h1 = _fc(nc, pools, xt_slice, KT1, aps["fc1_w4"], aps["fc1_b"],
                     i, HID, True, ones, "w4", FP8E4, 14,
                     inv_scale=1.0 / S1, dr=True)
        else:
            def xt_slice(k):
                return xt_tiles[k // 14][:, (k % 14) * 128:(k % 14) * 128 + R]

            h1 = _fc(nc, pools, xt_slice, KT1, aps["fc1_w8"], aps["fc1_b"],
                     i, HID, True, ones, "w1", FP8, CH1, inv_scale=1.0 / S1)
        h1T = _transpose_h(nc, pools, h1, ident)

        def h1t_slice(k):
            return h1T[:, k * 128:k * 128 + R]

        h2 = _fc(nc, pools, h1t_slice, KT2, aps["fc2_w8"], aps["fc2_b"],
                 i, HID, True, ones, "w2", FP8, KT2, inv_scale=1.0 / S2)
        h2T = _transpose_h(nc, pools, h2, ident)

        def h2t_slice(k):
            return h2T[:, k * 128:k * 128 + R]

        if want == "deltas":
            return _fc(nc, pools, h2t_slice, KT2, aps["reg_wb"],
                       aps["reg_b"], i, 4, False, ones, "wsm", BF16, KT2)
        logits = _fc(nc, pools, h2t_slice, KT2, aps["cls_wb"],
                     aps["cls_b"], i, NCLS, False, ones, "wsm", BF16, KT2)
        return _softmax(nc, pools, logits)

    acc = pools["acc"].tile([R, NCLS], F32, tag="acc", name="acc")
    xt_tiles = None
    for s in range(N_STAGES):
        idx_i32, wx_eff, wy_eff = _roi_prep(nc, pools, rois_t, grid_t)
        # stage 2 feeds only the three probs-path GEMMs; its pooled features
        # go straight to fp8e4 so those fc1 GEMMs can run DoubleRow
        xdt = BF16 if (s < 2 or not DR_ON) else FP8E4
        xt_tiles = [pools["xt"].tile([128, 14 * 128], xdt, tag=f"xt{s >= 2}{j}",
                                     name=f"xt{j}")
                    for j in range(POOL)]
        _gather_interp_transpose(nc, pools, aps["feats_b"], idx_i32, wx_eff,
                                 wy_eff, ident_b, xt_tiles)
        if s < 2:
            deltas = head_app(s, xt_tiles, "deltas")
            rois_t = _delta2bbox(nc, pools, rois_t, deltas, stds_t,
                                 pools["rois"])
        else:
            p3 = head_app(2, xt_tiles, "probs", dr=DR_ON)
            nc.vector.tensor_copy(acc[:], p3[:])

    for i in range(2):
        pi = head_app(i, xt_tiles, "probs", dr=DR_ON)
        nc.vector.tensor_tensor(acc[:], acc[:], pi[:], op=Alu.add)

    outp = pools["h"].tile([R, NCLS], F32, tag="outp", name="outp")
    nc.vector.tensor_scalar(outp[:], acc[:], 1.0 / 3.0, None, op0=Alu.mult)
    nc.sync.dma_start(aps["out"][:], outp[:])


# ---------------------------------------------------------------------------
# host side
# ---------------------------------------------------------------------------

_CACHE: dict = {}


def build_program(reps: int = 1):
    nc = bacc.Bacc("TRN2", target_bir_lowering=False, debug=False,
                   num_devices=N_CORES)
    aps = {
        "feats_b": nc.dram_tensor("feats_b", [FEAT_ROWS, 2 * C], BF16,
                                  kind="ExternalInput").ap(),
        "rois": nc.dram_tensor("rois", [R, 4], F32, kind="ExternalInput").ap(),
        "fc1_w8": nc.dram_tensor("fc1_w8", [3, KT1 // CH1, 128, CH1 * HID],
                                 FP8, kind="ExternalInput").ap(),
        "fc1_w4": nc.dram_tensor("fc1_w4", [3, KT1 // 14, 128, 14 * HID],
                                 FP8E4, kind="ExternalInput").ap(),
        "fc1_b": nc.dram_tensor("fc1_b", [3, HID], F32,
                                kind="ExternalInput").ap(),
        "fc2_w8": nc.dram_tensor("fc2_w8", [3, 1, 128, KT2 * HID], FP8,
                                 kind="ExternalInput").ap(),
        "fc2_b": nc.dram_tensor("fc2_b", [3, HID], F32,
                                kind="ExternalInput").ap(),
        "cls_wb": nc.dram_tensor("cls_wb", [3, 1, 128, KT2 * NCLS], BF16,
                                 kind="ExternalInput").ap(),
        "cls_b": nc.dram_tensor("cls_b", [3, NCLS], F32,
                                kind="ExternalInput").ap(),
        "reg_wb": nc.dram_tensor("reg_wb", [3, 1, 128, KT2 * 4], BF16,
                                 kind="ExternalInput").ap(),
        "reg_b": nc.dram_tensor("reg_b", [3, 4], F32,
                                kind="ExternalInput").ap(),
        "grid_c": nc.dram_tensor("grid_c", [128, POOL], F32,
                                 kind="ExternalInput").ap(),
        "stds_c": nc.dram_tensor("stds_c", [128, 4], F32,
                                 kind="ExternalInput").ap(),
        "out": nc.dram_tensor("out", [R, NCLS], F32,
                              kind="ExternalOutput").ap(),
    }
    with tile.TileContext(nc) as tc:
        for _ in range(reps):
            with ExitStack() as ctx:
                build_kernel(ctx, tc, aps)
    nc.compile()
    return nc


def host_consts():
    grid = ((np.arange(POOL, dtype=np.float32) + np.float32(0.5))
            / np.float32(POOL))
    grid_c = np.broadcast_to(grid, (128, POOL)).copy()
    stds_c = np.broadcast_to(
        np.array([0.1, 0.1, 0.2, 0.2], dtype=np.float32), (128, 4)).copy()
    return grid_c, stds_c


def _tile_w(w, kt, chunk, n_out):
    """[3, K, n_out] -> [3, K//(128*chunk), 128, chunk*n_out] k-tiled layout:
    row p of tile (c, j) holds w[c*chunk*128 + j*128 + p]."""
    w = np.ascontiguousarray(w)
    n_ch = kt // chunk
    wt = (w.reshape(3, n_ch, chunk, 128, n_out)
          .transpose(0, 1, 3, 2, 4)
          .reshape(3, n_ch, 128, chunk * n_out))
    return np.ascontiguousarray(wt)


def make_in_maps(inputs: dict) -> list:
    import ml_dtypes
    f32 = lambda x: np.ascontiguousarray(np.asarray(x, dtype=np.float32))
    bf16 = lambda x: np.ascontiguousarray(
        np.asarray(x, dtype=np.float32).astype(ml_dtypes.bfloat16))

    def fp8(x, scale):
        y = np.clip(np.asarray(x, np.float32) * scale, -15.0, 15.0)
        return np.ascontiguousarray(y.astype(ml_dtypes.float8_e3m4))

    def fp8e4(x, scale):
        y = np.clip(np.asarray(x, np.float32) * scale, -200.0, 200.0)
        return np.ascontiguousarray(y.astype(ml_dtypes.float8_e4m3))

    def pair_level(p):
        # [S, S, C] -> [S*S, 2C]: row (y,x) = [feat(y,x), feat(min(y+1,S-1),x)]
        f = f32(p)[0]
        fn = np.concatenate([f[1:], f[-1:]], axis=0)
        return np.concatenate([f, fn], axis=2).reshape(-1, 2 * C)

    feats = np.concatenate([pair_level(inputs[k])
                            for k in ("P2", "P3", "P4", "P5")], axis=0)
    feats = bf16(feats)
    grid_c, stds_c = host_consts()
    rois = f32(inputs["rois"])
    shared = {
        "feats_b": feats,
        "fc1_w8": _tile_w(fp8(inputs["fc1_w"], S1), KT1, CH1, HID),
        "fc1_w4": _tile_w(fp8e4(inputs["fc1_w"], S1), KT1, 14, HID),
        "fc1_b": f32(inputs["fc1_b"]) * np.float32(S1),
        "fc2_w8": _tile_w(fp8(inputs["fc2_w"], S2), KT2, KT2, HID),
        "fc2_b": f32(inputs["fc2_b"]) * np.float32(S2),
        "cls_wb": _tile_w(bf16(inputs["cls_w"]), KT2, KT2, NCLS),
        "cls_b": f32(inputs["cls_b"]),
        "reg_wb": _tile_w(bf16(inputs["reg_w"]), KT2, KT2, 4),
        "reg_b": f32(inputs["reg_b"]),
        "grid_c": grid_c, "stds_c": stds_c,
    }
    return [dict(shared, rois=rois[c * R:(c + 1) * R]) for c in range(N_CORES)]


def make_runner(nc):
    """Jitted SPMD executor: rois/outputs sharded over cores, all other
    inputs replicated (avoids the 8x host-side concat of the big weights)."""
    import jax
    from jax.sharding import Mesh, PartitionSpec
    from jax.experimental.shard_map import shard_map
    from concourse import bass2jax

    bass2jax.install_neuronx_cc_hook()
    pname = nc.partition_id_tensor.name if nc.partition_id_tensor else None
    in_names, out_names, out_avals = [], [], []
    for alloc in nc.m.functions[0].allocations:
        if not isinstance(alloc, mybir.MemoryLocationSet):
            continue
        name = alloc.memorylocations[0].name
        if alloc.kind == "ExternalInput":
            if name != pname:
                in_names.append(name)
        elif alloc.kind == "ExternalOutput":
            out_names.append(name)
            out_avals.append(jax.core.ShapedArray(
                tuple(alloc.tensor_shape), mybir.dt.np(alloc.dtype)))
    n_outs = len(out_avals)
    names_full = list(in_names) + out_names + ([pname] if pname else [])

    def _body(*args):
        ops = list(args)
        if pname is not None:
            ops.append(bass2jax.partition_id_tensor())
        return tuple(bass2jax._bass_exec_p.bind(
            *ops, out_avals=tuple(out_avals), in_names=tuple(names_full),
            out_names=tuple(out_names), lowering_input_output_aliases=(),
            sim_require_finite=True, sim_require_nnan=True, nc=nc))

    devices = jax.devices()[:N_CORES]
    mesh = Mesh(np.asarray(devices), ("core",))
    P_ = PartitionSpec
    in_specs = tuple(P_("core") if nm == "rois" else P_() for nm in in_names) \
        + (P_("core"),) * n_outs
    sharded = jax.jit(
        shard_map(_body, mesh=mesh, in_specs=in_specs,
                  out_specs=(P_("core"),) * n_outs, check_rep=False),
        keep_unused=True)

    def _args(shared, rois_full):
        args = [rois_full if nm == "rois" else shared[nm] for nm in in_names]
        args += [np.zeros((N_CORES * a.shape[0], *a.shape[1:]), a.dtype)
                 for a in out_avals]
        return args

    def prepare(shared: dict, rois_full: np.ndarray):
        from jax.sharding import NamedSharding
        args = _args(shared, rois_full)
        shards = [NamedSharding(mesh, s) for s in in_specs]
        return [jax.device_put(a, s) for a, s in zip(args, shards)]

    def run_dev(dev_args):
        out = sharded(*dev_args)
        jax.block_until_ready(out)
        return np.asarray(out[0])

    def run(shared: dict, rois_full: np.ndarray):
        out = sharded(*_args(shared, rois_full))
        jax.block_until_ready(out)
        return np.asarray(out[0])

    run.prepare = prepare
    run.run_dev = run_dev
    run.sharded = sharded
    return run


def kernel(**inputs) -> np.ndarray:
    if "nc" not in _CACHE:
        _CACHE["nc"] = build_program()
        _CACHE["run"] = make_runner(_CACHE["nc"])
    in_maps = make_in_maps(inputs)
    shared = dict(in_maps[0])
    rois_full = np.ascontiguousarray(np.asarray(inputs["rois"], np.float32))
    out = _CACHE["run"](shared, rois_full)
    return out.astype(np.float32)


# revision 29
# speedup vs baseline: 1.1185x; 1.1185x over previous
"""CascadeRCNN head (3-stage cascade + test-time ensemble) on 8 Trainium2 NeuronCores.

Data-parallel over rois: 1000 rois sharded 8 x 125; FPN feature maps and head
weights replicated. Everything (ROIAlign gather+bilinear, FC GEMMs, softmax,
delta2bbox) runs on-device; host only shards/concats.

Precision plan (tolerance 2e-2; measured headroom ~45x at fp32/bf16 baseline):
 - FPN features: bf16 (halves the gather traffic).
 - fc1/fc2 weights: fp8 e3m4, scaled by 128/64 (power of two, exact descale
   folded into the post-GEMM activation). PSUM accumulation stays fp32.
 - cls/reg weights: bf16. Activations x/h1T/h2T: bf16 lhsT, fp32 psum.
This cuts per-core HBM traffic from ~265MB to ~109MB; fc1 weight streams are
loaded in 7-k-tile (896KB) chunks to stay transfer-bound, not issue-bound.
"""

import numpy as np
from contextlib import ExitStack

import concourse.bass as bass
import concourse.tile as tile
from concourse import bacc, mybir
from concourse.masks import make_identity

F32 = mybir.dt.float32
BF16 = mybir.dt.bfloat16
FP8 = mybir.dt.float8e3
I32 = mybir.dt.int32
Alu = mybir.AluOpType
Act = mybir.ActivationFunctionType

N_CORES = 8
R = 125              # rois per core
POOL = 7
C = 256
K1 = 12544           # 7*7*256
KT1 = 98             # fc1 k-tiles
CH1 = 7              # fc1 k-tiles per weight DMA
HID = 1024
KT2 = 8
NCLS = 81
IMG = 1024.0
S1 = 128.0           # fc1 weight fp8 scale (power of two)
S2 = 64.0            # fc2 weight fp8 scale

# concatenated feature table: P2 (256x256), P3 (128x128), P4 (64x64), P5 (32x32)
FEAT_ROWS = 256 * 256 + 128 * 128 + 64 * 64 + 32 * 32  # 87040

N_STAGES = 3


def _roi_prep(nc, pools, rois_t, grid_t):
    """From rois [R,4] compute gather indices and bilinear weights.

    Returns (idx_i32 [R, 98], wx_eff [R,7], wy_eff [R,7]).
    idx free layout: (jy, jx, yn) -> col jy*14 + jx*2 + yn. Each index is a row
    of the feats table; a gather of 512 elems covers pixel columns (bx, bx+1).
    """
    prep = pools["prep"]
    v = nc.vector

    def pt(cols, dtype=F32, tag=None):
        return prep.tile([R, cols], dtype, tag=tag, name=tag)

    y1 = rois_t[:, 0:1]
    x1 = rois_t[:, 1:2]
    y2 = rois_t[:, 2:3]
    x2 = rois_t[:, 3:4]

    hh = pt(1, tag="hh"); v.tensor_tensor(hh[:], y2, y1, op=Alu.subtract)
    ww = pt(1, tag="ww"); v.tensor_tensor(ww[:], x2, x1, op=Alu.subtract)
    hw = pt(1, tag="hw"); v.tensor_tensor(hw[:], hh[:], ww[:], op=Alu.mult)
    v.tensor_scalar(hw[:], hw[:], 1e-6, None, op0=Alu.max)

    # level selection: lvl = 2 + (hw>=112^2) + (hw>=224^2) + (hw>=448^2)
    g2 = pt(1, tag="g2"); v.tensor_scalar(g2[:], hw[:], 12544.0, None, op0=Alu.is_ge)
    g3 = pt(1, tag="g3"); v.tensor_scalar(g3[:], hw[:], 50176.0, None, op0=Alu.is_ge)
    g4 = pt(1, tag="g4"); v.tensor_scalar(g4[:], hw[:], 200704.0, None, op0=Alu.is_ge)

    # inv_stride = 0.25 - 0.125*g2 - 0.0625*g3 - 0.03125*g4  (exact)
    invs = pt(1, tag="invs")
    v.tensor_scalar(invs[:], g2[:], -0.125, 0.25, op0=Alu.mult, op1=Alu.add)
    t0 = pt(1, tag="t0")
    v.tensor_scalar(t0[:], g3[:], -0.0625, None, op0=Alu.mult)
    v.tensor_tensor(invs[:], invs[:], t0[:], op=Alu.add)
    v.tensor_scalar(t0[:], g4[:], -0.03125, None, op0=Alu.mult)
    v.tensor_tensor(invs[:], invs[:], t0[:], op=Alu.add)

    # feature side S = 1024 * inv_stride in {256,128,64,32}; level base offset
    S = pt(1, tag="S"); v.tensor_scalar(S[:], invs[:], 1024.0, None, op0=Alu.mult)
    base = pt(1, tag="base")
    v.tensor_scalar(base[:], g2[:], 65536.0, None, op0=Alu.mult)
    v.tensor_scalar(t0[:], g3[:], 16384.0, None, op0=Alu.mult)
    v.tensor_tensor(base[:], base[:], t0[:], op=Alu.add)
    v.tensor_scalar(t0[:], g4[:], 4096.0, None, op0=Alu.mult)
    v.tensor_tensor(base[:], base[:], t0[:], op=Alu.add)
    Sm1 = pt(1, tag="Sm1"); v.tensor_scalar(Sm1[:], S[:], -1.0, None, op0=Alu.add)
    Sm2 = pt(1, tag="Sm2"); v.tensor_scalar(Sm2[:], S[:], -2.0, None, op0=Alu.add)

    # scaled roi coords (exact: multiply by power of two)
    sy1 = pt(1, tag="sy1"); v.tensor_tensor(sy1[:], y1, invs[:], op=Alu.mult)
    sx1 = pt(1, tag="sx1"); v.tensor_tensor(sx1[:], x1, invs[:], op=Alu.mult)
    sy2 = pt(1, tag="sy2"); v.tensor_tensor(sy2[:], y2, invs[:], op=Alu.mult)
    sx2 = pt(1, tag="sx2"); v.tensor_tensor(sx2[:], x2, invs[:], op=Alu.mult)
    dy = pt(1, tag="dy"); v.tensor_tensor(dy[:], sy2[:], sy1[:], op=Alu.subtract)
    dx = pt(1, tag="dx"); v.tensor_tensor(dx[:], sx2[:], sx1[:], op=Alu.subtract)

    def axis_prep(scoord, dcoord, suffix, clamp):
        # ys = grid*d + s  (matches ref rounding: mult then add)
        ys = pt(POOL, tag="ys" + suffix)
        v.tensor_scalar(ys[:], grid_t[0:R, :], dcoord[:], scoord[:],
                        op0=Alu.mult, op1=Alu.add)
        # robust floor (works under truncation or round-to-nearest casts)
        yi = pt(POOL, I32, tag="yi" + suffix)
        v.tensor_copy(yi[:], ys[:])
        yf = pt(POOL, tag="yf" + suffix)
        v.tensor_copy(yf[:], yi[:])
        gt = pt(POOL, tag="gt" + suffix)
        v.tensor_tensor(gt[:], yf[:], ys[:], op=Alu.is_gt)
        y0f = pt(POOL, tag="y0f" + suffix)
        v.tensor_tensor(y0f[:], yf[:], gt[:], op=Alu.subtract)
        # clip to [0, S-1] (ys >= 0 so lower clip is a no-op)
        y0c = pt(POOL, tag="y0c" + suffix)
        v.tensor_scalar(y0c[:], y0f[:], Sm1[:], None, op0=Alu.min)
        # index clamp: x needs bx+1 <= S-1 (contiguous pair read); y is baked
        # into the paired table, so its row index is y0c itself.
        by = pt(POOL, tag="by" + suffix)
        v.tensor_scalar(by[:], y0c[:], clamp[:], None, op0=Alu.min)
        # weight: clip(ys - y0c, 0, 1); force 1.0 at the right edge
        wy = pt(POOL, tag="wy" + suffix)
        v.tensor_tensor(wy[:], ys[:], y0c[:], op=Alu.subtract)
        v.tensor_scalar(wy[:], wy[:], 0.0, 1.0, op0=Alu.max, op1=Alu.min)
        fl = pt(POOL, tag="fl" + suffix)
        v.tensor_scalar(fl[:], y0f[:], Sm1[:], None, op0=Alu.is_ge)
        v.tensor_tensor(wy[:], wy[:], fl[:], op=Alu.max)
        return by, wy

    by, wy_eff = axis_prep(sy1, dy, "Y", Sm1)
    bx, wx_eff = axis_prep(sx1, dx, "X", Sm2)

    # rowA = base + by*S  [R,7]
    rowA = pt(POOL, tag="rowA")
    v.tensor_scalar(rowA[:], by[:], S[:], base[:], op0=Alu.mult, op1=Alu.add)

    # idx layout [R, 7jy, 7jx]; one paired-table row index per bin. The
    # paired table row (y, x) holds [feat(y, x), feat(min(y+1, S-1), x)], so a
    # 1024-elem read from row (base + y*S + x) covers all 4 bilinear
    # neighbors: [f(y,x0), f(y+1,x0), f(y,x0+1), f(y+1,x0+1)].
    idxf = pools["idx"].tile([R, POOL * POOL], F32, tag="idxf", name="idxf")
    v3 = idxf[:].rearrange("p (a b) -> p a b", a=POOL)
    v.tensor_tensor(v3[:, :, :],
                    rowA[:, :, None].to_broadcast([R, POOL, POOL]),
                    bx[:, None, :].to_broadcast([R, POOL, POOL]),
                    op=Alu.add)
    idx_i32 = pools["idx"].tile([R, POOL * POOL], I32, tag="idxi", name="idxi")
    v.tensor_copy(idx_i32[:], idxf[:])
    return idx_i32, wx_eff, wy_eff


def _gather_interp_transpose(nc, pools, feats_ap, idx_i32, wx_eff, wy_eff,
                             ident_b, xt_tiles):
    """Fill xt_tiles (7 tiles of [128, 14*128]; K-block k at cols
    (k%14)*128, 125 used) with pooled features transposed (x^T for fc1).
    The psum->SBUF copy converts to each xt tile's dtype (bf16 or fp8e4)."""
    v = nc.vector
    # Per-stage x-interp diagonal weight tiles: Wd[jx][0] = diag(1-wx[:,jx]),
    # Wd[jx][1] = diag(wx[:,jx]). The x-interp then folds into the PE
    # "transposes" as psum-accumulated diagonal matmuls:
    #   xt[c, roi] = (1-wx)(roi)*T0[roi, c] + wx(roi)*T1[roi, c]
    wxm = pools["prep"].tile([R, POOL], F32, tag="wxm", name="wxm")
    v.tensor_scalar(wxm[:], wx_eff[:], -1.0, 1.0, op0=Alu.mult, op1=Alu.add)
    wxd = pools["wxd"].tile([R, POOL * 2 * R], BF16, tag="wxd", name="wxd")
    wdv = wxd[:].rearrange("p (b x q) -> p b x q", b=POOL, x=2)
    for jx in range(POOL):
        v.tensor_scalar(wdv[:, jx, 0, :], ident_b[0:R, 0:R],
                        wxm[:, jx:jx + 1], None, op0=Alu.mult)
        v.tensor_scalar(wdv[:, jx, 1, :], ident_b[0:R, 0:R],
                        wx_eff[:, jx:jx + 1], None, op0=Alu.mult)
    for jy in range(POOL):
        G = pools["gath"].tile([R, 7 * 1024], FP8, tag="G", name="G")
        # HW indirect DMA consumes exactly one index per output partition,
        # so issue one gather per jx bin; the paired table packs all 4
        # bilinear neighbors into one 1024-elem contiguous read.
        for j in range(POOL):
            nc.gpsimd.indirect_dma_start(
                out=G[:, j * 1024:(j + 1) * 1024],
                out_offset=None,
                in_=feats_ap[:],
                in_offset=bass.IndirectOffsetOnAxis(
                    ap=idx_i32[:, jy * POOL + j:jy * POOL + j + 1], axis=0),
            )
        # layout [R, jx, xn, yn, c]; interp math lands in a bf16 tile so
        # no intermediate is ever quantized to fp8
        gv = G[:].rearrange("p (b x y e) -> p b x y e", b=POOL, x=2, y=2)
        T = pools["yint"].tile([R, POOL * 2 * 256], BF16, tag="T", name="T")
        tv = T[:].rearrange("p (b x e) -> p b x e", b=POOL, x=2)
        # y-interp: T = G0 + wy*(G1 - G0)
        v.tensor_tensor(tv, gv[:, :, :, 1, :], gv[:, :, :, 0, :],
                        op=Alu.subtract)
        nc.scalar.mul(tv, tv, wy_eff[:, jy:jy + 1])
        v.tensor_tensor(tv, tv, gv[:, :, :, 0, :], op=Alu.add)
        # x-interp + transpose fused on the PE: per (jx, c-half) accumulate
        # the two xn diagonal matmuls into one psum slot; two slots share a
        # PSUM bank so each psum->SBUF copy moves a pair.
        xt = xt_tiles[jy]
        xv = xt[:].rearrange("p (t c) -> p t c", c=128)
        for jx in range(POOL):
            ps = pools["pt"].tile([128, 2, 128], F32, space="PSUM",
                                  tag="ptrf", name="ptrf")
            for u in range(2):
                for xn in range(2):
                    nc.tensor.matmul(
                        ps[:, u, 0:R],
                        lhsT=tv[:, jx, xn, u * 128:(u + 1) * 128],
                        rhs=wdv[:, jx, xn, :],
                        start=(xn == 0), stop=(xn == 1))
            nc.scalar.copy(xv[:, 2 * jx:2 * jx + 2, 0:R], ps[:, :, 0:R])


def _fc(nc, pools, xt_slices, KT, w_tiled, b2, head, n_out, relu, ones, w_tag,
        wdt, chunk, inv_scale=1.0, h_dt=F32):
    """h[R, n_out] = act((x @ W)*inv_scale + b*inv_scale).

    W pre-tiled on host as [3, KT//chunk, 128, chunk*n_out]; b pre-scaled by
    1/inv_scale so the single post-GEMM activation descale is exact.
    xt_slices(k) -> lhsT AP [128, R]. PSUM matmul outputs must stay within
    one 2KB bank, hence the 512-column psum split."""
    offs = list(range(0, n_out, 512))
    sizes = [min(512, n_out - o) for o in offs]
    psums = [pools["pfc"].tile([R, 512], F32, space="PSUM", tag=f"ps{j}",
                               name=f"ps{j}")
             for j in range(len(offs))]
    dma_eng = nc.sync if w_tag == "w1" else nc.scalar
    n_ch = KT // chunk
    for c in range(n_ch):
        wt = pools[w_tag].tile([128, chunk * n_out], wdt, tag=w_tag, name=w_tag)
        dma_eng.dma_start(wt[:], w_tiled[head, c])
        for j in range(chunk):
            k = c * chunk + j
            for jj, (o, sz) in enumerate(zip(offs, sizes)):
                nc.tensor.matmul(psums[jj][:, 0:sz], lhsT=xt_slices(k),
                                 rhs=wt[:, j * n_out + o:j * n_out + o + sz],
                                 start=(k == 0), stop=False)
    bt = pools["bias"].tile([1, n_out], F32, tag="bias", name="bias")
    nc.scalar.dma_start(bt[:], b2[head][None, :])
    for jj, (o, sz) in enumerate(zip(offs, sizes)):
        nc.tensor.matmul(psums[jj][:, 0:sz], lhsT=ones[0:1, 0:R],
                         rhs=bt[0:1, o:o + sz], start=False, stop=True)
    h = pools["h"].tile([R, n_out], h_dt, tag=f"h{n_out}", name=f"h{n_out}")
    for jj, (o, sz) in enumerate(zip(offs, sizes)):
        if relu:
            nc.scalar.activation(h[:, o:o + sz], psums[jj][:, 0:sz], Act.Relu,
                                 scale=inv_scale)
        elif inv_scale != 1.0:
            nc.scalar.activation(h[:, o:o + sz], psums[jj][:, 0:sz], Act.Copy,
                                 scale=inv_scale)
        else:
            nc.scalar.copy(h[:, o:o + sz], psums[jj][:, 0:sz])
    return h


def _transpose_h(nc, pools, h, ident):
    """h [R, 1024] fp32 -> hT [128, 1024] bf16 (8 blocks, 125 cols used)."""
    hT = pools["ht"].tile([128, HID], BF16, tag="hT", name="hT")
    hv = hT[:].rearrange("p (t c) -> p t c", c=128)
    for t in range(0, HID // 128, 2):
        ps = pools["pt"].tile([128, 2, 128], F32, space="PSUM", tag="ptrf",
                              name="ptrf")
        for u in range(2):
            nc.tensor.transpose(out=ps[:, u, 0:R],
                                in_=h[:, (t + u) * 128:(t + u + 1) * 128],
                                identity=ident[0:R, 0:R])
        nc.scalar.copy(hv[:, t:t + 2, 0:R], ps[:, :, 0:R])
    return hT


def _softmax(nc, pools, logits):
    v = nc.vector
    rmax = pools["prep"].tile([R, 1], F32, tag="rmax", name="rmax")
    v.tensor_reduce(rmax[:], logits[:], axis=mybir.AxisListType.X, op=Alu.max)
    nmax = pools["prep"].tile([R, 1], F32, tag="nmax", name="nmax")
    v.tensor_scalar(nmax[:], rmax[:], -1.0, None, op0=Alu.mult)
    e = pools["h"].tile([R, NCLS], F32, tag="smx", name="smx")
    nc.scalar.activation(e[:], logits[:], Act.Exp, bias=nmax[:], scale=1.0)
    ssum = pools["prep"].tile([R, 1], F32, tag="ssum", name="ssum")
    v.tensor_reduce(ssum[:], e[:], axis=mybir.AxisListType.X, op=Alu.add)
    rsum = pools["prep"].tile([R, 1], F32, tag="rsum", name="rsum")
    v.reciprocal(rsum[:], ssum[:])
    v.tensor_scalar(e[:], e[:], rsum[:], None, op0=Alu.mult)
    return e


def _delta2bbox(nc, pools, rois_t, deltas, stds_t, rois_pool):
    """rois_next = delta2bbox(rois_t, deltas) following the reference op order."""
    v = nc.vector
    prep = pools["prep"]

    def pt(tag):
        return prep.tile([R, 1], F32, tag=tag, name=tag)

    d = prep.tile([R, 4], F32, tag="dsc", name="dsc")
    v.tensor_tensor(d[:], deltas[:], stds_t[0:R, :], op=Alu.mult)
    y1 = rois_t[:, 0:1]; x1 = rois_t[:, 1:2]; y2 = rois_t[:, 2:3]; x2 = rois_t[:, 3:4]
    hh = pt("b_h"); v.tensor_tensor(hh[:], y2, y1, op=Alu.subtract)
    ww = pt("b_w"); v.tensor_tensor(ww[:], x2, x1, op=Alu.subtract)
    # cy = (y1 + 0.5*h) + d0*h ; cx likewise
    hh2 = pt("b_h2"); v.tensor_scalar(hh2[:], hh[:], 0.5, None, op0=Alu.mult)
    cy = pt("b_cy"); v.tensor_tensor(cy[:], y1, hh2[:], op=Alu.add)
    t = pt("b_t"); v.tensor_tensor(t[:], d[:, 0:1], hh[:], op=Alu.mult)
    v.tensor_tensor(cy[:], cy[:], t[:], op=Alu.add)
    ww2 = pt("b_w2"); v.tensor_scalar(ww2[:], ww[:], 0.5, None, op0=Alu.mult)
    cx = pt("b_cx"); v.tensor_tensor(cx[:], x1, ww2[:], op=Alu.add)
    v.tensor_tensor(t[:], d[:, 1:2], ww[:], op=Alu.mult)
    v.tensor_tensor(cx[:], cx[:], t[:], op=Alu.add)
    # h' = h*exp(d2), w' = w*exp(d3)
    eh = pt("b_eh"); nc.scalar.activation(eh[:], d[:, 2:3], Act.Exp)
    ew = pt("b_ew"); nc.scalar.activation(ew[:], d[:, 3:4], Act.Exp)
    v.tensor_tensor(hh[:], hh[:], eh[:], op=Alu.mult)
    v.tensor_tensor(ww[:], ww[:], ew[:], op=Alu.mult)
    v.tensor_scalar(hh2[:], hh[:], 0.5, None, op0=Alu.mult)
    v.tensor_scalar(ww2[:], ww[:], 0.5, None, op0=Alu.mult)
    rn = rois_pool.tile([R, 4], F32, tag="rois", name="rois")
    v.tensor_tensor(rn[:, 0:1], cy[:], hh2[:], op=Alu.subtract)
    v.tensor_tensor(rn[:, 1:2], cx[:], ww2[:], op=Alu.subtract)
    v.tensor_tensor(rn[:, 2:3], cy[:], hh2[:], op=Alu.add)
    v.tensor_tensor(rn[:, 3:4], cx[:], ww2[:], op=Alu.add)
    for j in range(4):
        v.tensor_scalar(rn[:, j:j + 1], rn[:, j:j + 1], 0.0, IMG,
                        op0=Alu.max, op1=Alu.min)
    return rn


def build_kernel(ctx: ExitStack, tc: "tile.TileContext", aps: dict):
    nc = tc.nc
    pools = {}
    for name, bufs, space in [
        ("const", 1, "SBUF"), ("rois", 2, "SBUF"), ("prep", 2, "SBUF"),
        ("idx", 2, "SBUF"), ("gath", 4, "SBUF"), ("yint", 4, "SBUF"),
        ("wxd", 2, "SBUF"),
        ("xt", 1, "SBUF"), ("w1", 4, "SBUF"), ("w2", 2, "SBUF"),
        ("wsm", 2, "SBUF"), ("bias", 2, "SBUF"), ("h", 2, "SBUF"),
        ("ht", 2, "SBUF"), ("acc", 1, "SBUF"),
        ("pt", 3, "PSUM"), ("pfc", 2, "PSUM"),
    ]:
        pools[name] = ctx.enter_context(tc.tile_pool(name=name, bufs=bufs,
                                                     space=space))

    ident = pools["const"].tile([128, 128], F32, tag="ident", name="ident")
    make_identity(nc, ident[:])
    ident_b = pools["const"].tile([128, 128], BF16, tag="identb", name="identb")
    nc.vector.tensor_copy(ident_b[:], ident[:])
    ones = pools["const"].tile([1, 128], F32, tag="ones", name="ones")
    nc.vector.memset(ones[:], 1.0)
    grid_t = pools["const"].tile([128, POOL], F32, tag="grid", name="grid")
    nc.sync.dma_start(grid_t[:], aps["grid_c"][:])
    stds_t = pools["const"].tile([128, 4], F32, tag="stds", name="stds")
    nc.sync.dma_start(stds_t[:], aps["stds_c"][:])

    rois_t = pools["rois"].tile([R, 4], F32, tag="rois", name="rois")
    nc.sync.dma_start(rois_t[:], aps["rois"][:])

    def head_app(i, xt_tiles, want):
        def xt_slice(k):
            return xt_tiles[k // 14][:, (k % 14) * 128:(k % 14) * 128 + R]

        h1 = _fc(nc, pools, xt_slice, KT1, aps["fc1_w8"], aps["fc1_b"],
                 i, HID, True, ones, "w1", FP8, CH1, inv_scale=1.0 / S1)
        h1T = _transpose_h(nc, pools, h1, ident)

        def h1t_slice(k):
            return h1T[:, k * 128:k * 128 + R]

        h2 = _fc(nc, pools, h1t_slice, KT2, aps["fc2_w8"], aps["fc2_b"],
                 i, HID, True, ones, "w2", FP8, KT2, inv_scale=1.0 / S2)
        h2T = _transpose_h(nc, pools, h2, ident)

        def h2t_slice(k):
            return h2T[:, k * 128:k * 128 + R]

        if want == "deltas":
            return _fc(nc, pools, h2t_slice, KT2, aps["reg_wb"],
                       aps["reg_b"], i, 4, False, ones, "wsm", BF16, KT2)
        logits = _fc(nc, pools, h2t_slice, KT2, aps["cls_wb"],
                     aps["cls_b"], i, NCLS, False, ones, "wsm", BF16, KT2)
        return _softmax(nc, pools, logits)

    acc = pools["acc"].tile([R, NCLS], F32, tag="acc", name="acc")
    xt_tiles = None
    for s in range(N_STAGES):
        idx_i32, wx_eff, wy_eff = _roi_prep(nc, pools, rois_t, grid_t)
        xt_tiles = [pools["xt"].tile([128, 14 * 128], BF16, tag=f"xt{j}",
                                     name=f"xt{j}")
                    for j in range(POOL)]
        _gather_interp_transpose(nc, pools, aps["feats_b"], idx_i32, wx_eff,
                                 wy_eff, ident_b, xt_tiles)
        if s < 2:
            deltas = head_app(s, xt_tiles, "deltas")
            rois_t = _delta2bbox(nc, pools, rois_t, deltas, stds_t,
                                 pools["rois"])
        else:
            p3 = head_app(2, xt_tiles, "probs")
            nc.vector.tensor_copy(acc[:], p3[:])

    for i in range(2):
        pi = head_app(i, xt_tiles, "probs")
        nc.vector.tensor_tensor(acc[:], acc[:], pi[:], op=Alu.add)

    outp = pools["h"].tile([R, NCLS], F32, tag="outp", name="outp")
    nc.vector.tensor_scalar(outp[:], acc[:], 1.0 / 3.0, None, op0=Alu.mult)
    nc.sync.dma_start(aps["out"][:], outp[:])


# ---------------------------------------------------------------------------
# host side
# ---------------------------------------------------------------------------

_CACHE: dict = {}


def build_program(reps: int = 1):
    nc = bacc.Bacc("TRN2", target_bir_lowering=False, debug=False,
                   num_devices=N_CORES)
    aps = {
        "feats_b": nc.dram_tensor("feats_b", [FEAT_ROWS, 2 * C], FP8,
                                  kind="ExternalInput").ap(),
        "rois": nc.dram_tensor("rois", [R, 4], F32, kind="ExternalInput").ap(),
        "fc1_w8": nc.dram_tensor("fc1_w8", [3, KT1 // CH1, 128, CH1 * HID],
                                 FP8, kind="ExternalInput").ap(),
        "fc1_b": nc.dram_tensor("fc1_b", [3, HID], F32,
                                kind="ExternalInput").ap(),
        "fc2_w8": nc.dram_tensor("fc2_w8", [3, 1, 128, KT2 * HID], FP8,
                                 kind="ExternalInput").ap(),
        "fc2_b": nc.dram_tensor("fc2_b", [3, HID], F32,
                                kind="ExternalInput").ap(),
        "cls_wb": nc.dram_tensor("cls_wb", [3, 1, 128, KT2 * NCLS], BF16,
                                 kind="ExternalInput").ap(),
        "cls_b": nc.dram_tensor("cls_b", [3, NCLS], F32,
                                kind="ExternalInput").ap(),
        "reg_wb": nc.dram_tensor("reg_wb", [3, 1, 128, KT2 * 4], BF16,
                                 kind="ExternalInput").ap(),
        "reg_b": nc.dram_tensor("reg_b", [3, 4], F32,
                                kind="ExternalInput").ap(),
        "grid_c": nc.dram_tensor("grid_c", [128, POOL], F32,
                                 kind="ExternalInput").ap(),
        "stds_c": nc.dram_tensor("stds_c", [128, 4], F32,
                                 kind="ExternalInput").ap(),
        "out": nc.dram_tensor("out", [R, NCLS], F32,
                              kind="ExternalOutput").ap(),
    }
    with tile.TileContext(nc) as tc:
        for _ in range(reps):
            with ExitStack() as ctx:
                build_kernel(ctx, tc, aps)
    nc.compile()
    return nc


def host_consts():
    grid = ((np.arange(POOL, dtype=np.float32) + np.float32(0.5))
            / np.float32(POOL))
    grid_c = np.broadcast_to(grid, (128, POOL)).copy()
    stds_c = np.broadcast_to(
        np.array([0.1, 0.1, 0.2, 0.2], dtype=np.float32), (128, 4)).copy()
    return grid_c, stds_c


def _tile_w(w, kt, chunk, n_out):
    """[3, K, n_out] -> [3, K//(128*chunk), 128, chunk*n_out] k-tiled layout:
    row p of tile (c, j) holds w[c*chunk*128 + j*128 + p]."""
    w = np.ascontiguousarray(w)
    n_ch = kt // chunk
    wt = (w.reshape(3, n_ch, chunk, 128, n_out)
          .transpose(0, 1, 3, 2, 4)
          .reshape(3, n_ch, 128, chunk * n_out))
    return np.ascontiguousarray(wt)


def make_in_maps(inputs: dict) -> list:
    import ml_dtypes
    f32 = lambda x: np.ascontiguousarray(np.asarray(x, dtype=np.float32))
    bf16 = lambda x: np.ascontiguousarray(
        np.asarray(x, dtype=np.float32).astype(ml_dtypes.bfloat16))

    def fp8(x, scale):
        y = np.clip(np.asarray(x, np.float32) * scale, -15.0, 15.0)
        return np.ascontiguousarray(y.astype(ml_dtypes.float8_e3m4))

    def pair_level(p):
        # [S, S, C] -> [S*S, 2C]: row (y,x) = [feat(y,x), feat(min(y+1,S-1),x)]
        f = f32(p)[0]
        fn = np.concatenate([f[1:], f[-1:]], axis=0)
        return np.concatenate([f, fn], axis=2).reshape(-1, 2 * C)

    feats = np.concatenate([pair_level(inputs[k])
                            for k in ("P2", "P3", "P4", "P5")], axis=0)
    feats = np.ascontiguousarray(
        np.clip(feats, -15.0, 15.0).astype(ml_dtypes.float8_e3m4))
    grid_c, stds_c = host_consts()
    rois = f32(inputs["rois"])
    shared = {
        "feats_b": feats,
        "fc1_w8": _tile_w(fp8(inputs["fc1_w"], S1), KT1, CH1, HID),
        "fc1_b": f32(inputs["fc1_b"]) * np.float32(S1),
        "fc2_w8": _tile_w(fp8(inputs["fc2_w"], S2), KT2, KT2, HID),
        "fc2_b": f32(inputs["fc2_b"]) * np.float32(S2),
        "cls_wb": _tile_w(bf16(inputs["cls_w"]), KT2, KT2, NCLS),
        "cls_b": f32(inputs["cls_b"]),
        "reg_wb": _tile_w(bf16(inputs["reg_w"]), KT2, KT2, 4),
        "reg_b": f32(inputs["reg_b"]),
        "grid_c": grid_c, "stds_c": stds_c,
    }
    return [dict(shared, rois=rois[c * R:(c + 1) * R]) for c in range(N_CORES)]


def make_runner(nc):
    """Jitted SPMD executor: rois/outputs sharded over cores, all other
    inputs replicated (avoids the 8x host-side concat of the big weights)."""
    import jax
    from jax.sharding import Mesh, PartitionSpec
    from jax.experimental.shard_map import shard_map
    from concourse import bass2jax

    bass2jax.install_neuronx_cc_hook()
    pname = nc.partition_id_tensor.name if nc.partition_id_tensor else None
    in_names, out_names, out_avals = [], [], []
    for alloc in nc.m.functions[0].allocations:
        if not isinstance(alloc, mybir.MemoryLocationSet):
            continue
        name = alloc.memorylocations[0].name
        if alloc.kind == "ExternalInput":
            if name != pname:
                in_names.append(name)
        elif alloc.kind == "ExternalOutput":
            out_names.append(name)
            out_avals.append(jax.core.ShapedArray(
                tuple(alloc.tensor_shape), mybir.dt.np(alloc.dtype)))
    n_outs = len(out_avals)
    names_full = list(in_names) + out_names + ([pname] if pname else [])

    def _body(*args):
        ops = list(args)
        if pname is not None:
            ops.append(bass2jax.partition_id_tensor())
        return tuple(bass2jax._bass_exec_p.bind(
            *ops, out_avals=tuple(out_avals), in_names=tuple(names_full),
            out_names=tuple(out_names), lowering_input_output_aliases=(),
            sim_require_finite=True, sim_require_nnan=True, nc=nc))

    devices = jax.devices()[:N_CORES]
    mesh = Mesh(np.asarray(devices), ("core",))
    P_ = PartitionSpec
    in_specs = tuple(P_("core") if nm == "rois" else P_() for nm in in_names) \
        + (P_("core"),) * n_outs
    sharded = jax.jit(
        shard_map(_body, mesh=mesh, in_specs=in_specs,
                  out_specs=(P_("core"),) * n_outs, check_rep=False),
        keep_unused=True)

    def _args(shared, rois_full):
        args = [rois_full if nm == "rois" else shared[nm] for nm in in_names]
        args += [np.zeros((N_CORES * a.shape[0], *a.shape[1:]), a.dtype)
                 for a in out_avals]
        return args

    def prepare(shared: dict, rois_full: np.ndarray):
        from jax.sharding import NamedSharding
        args = _args(shared, rois_full)
        shards = [NamedSharding(mesh, s) for s in in_specs]
        return [jax.device_put(a, s) for a, s in zip(args, shards)]

    def run_dev(dev_args):
        out = sharded(*dev_args)
        jax.block_until_ready(out)
        return np.asarray(out[0])

    def run(shared: dict, rois_full: np.ndarray):
        out = sharded(*_args(shared, rois_full))
        jax.block_until_ready(out)
        return np.asarray(out[0])

    run.prepare = prepare
    run.run_dev = run_dev
    run.sharded = sharded
    return run


def kernel(**inputs) -> np.ndarray:
    if "nc" not in _CACHE:
        _CACHE["nc"] = build_program()
        _CACHE["run"] = make_runner(_CACHE["nc"])
    in_maps = make_in_maps(inputs)
    shared = dict(in_maps[0])
    rois_full = np.ascontiguousarray(np.asarray(inputs["rois"], np.float32))
    out = _CACHE["run"](shared, rois_full)
    return out.astype(np.float32)


# revision 31
# speedup vs baseline: 1.4682x; 1.3126x over previous
"""CascadeRCNN head (3-stage cascade + test-time ensemble) on 8 Trainium2 NeuronCores.

Data-parallel over rois: 1000 rois sharded 8 x 125; FPN feature maps and head
weights replicated. Everything (ROIAlign gather+bilinear, FC GEMMs, softmax,
delta2bbox) runs on-device; host only shards/concats.

Optimizations vs the naive structure (tolerance 2e-2, measured ~2.7e-3):
 - Vertically-paired feature table in HBM: row (y,x) holds pixels (y,x) and
   (y+1,x), so ONE 1024-elem indirect gather fetches all 4 bilinear
   neighbors of a bin: 49 gather calls/stage instead of 98+ (the ~1us SWDGE
   issue cost per indirect DMA was a serial gpsimd bottleneck).
 - fp8 e3m4 (4 mantissa bits) for the FPN features and the fc1/fc2 weights;
   weights scaled by 128/64 (powers of two; exact descale folds into the
   post-GEMM Relu via activation scale). PSUM accumulation stays fp32; all
   interp math lands in bf16 tiles (no fp8 intermediates). cls/reg bf16.
   Per-core HBM traffic drops ~265MB -> ~88MB.
 - x-interp is fused into the PE transposes: two diagonal-weight matmuls
   (rhs = diag(1-wx), diag(wx)) accumulate per psum slot, replacing three
   DVE passes per bin-row; psum pairs share a bank so each psum->SBUF copy
   moves two K-blocks.
 - fc1 weights stream in 7-k-tile (896KB) chunks on the sync queue;
   4-deep gather/interp/weight pools keep the per-jy pipeline full.
(Note: PE DoubleRow fp8 was tried and wedges the device - do not re-enable.)
"""

import numpy as np
from contextlib import ExitStack

import concourse.bass as bass
import concourse.tile as tile
from concourse import bacc, mybir
from concourse.masks import make_identity

F32 = mybir.dt.float32
BF16 = mybir.dt.bfloat16
FP8 = mybir.dt.float8e3
I32 = mybir.dt.int32
Alu = mybir.AluOpType
Act = mybir.ActivationFunctionType

N_CORES = 8
R = 125              # rois per core
POOL = 7
C = 256
K1 = 12544           # 7*7*256
KT1 = 98             # fc1 k-tiles
CH1 = 7              # fc1 k-tiles per weight DMA
HID = 1024
KT2 = 8
NCLS = 81
IMG = 1024.0
S1 = 128.0           # fc1 weight fp8 scale (power of two)
S2 = 64.0            # fc2 weight fp8 scale

# concatenated feature table: P2 (256x256), P3 (128x128), P4 (64x64), P5 (32x32)
FEAT_ROWS = 256 * 256 + 128 * 128 + 64 * 64 + 32 * 32  # 87040

N_STAGES = 3


def _roi_prep(nc, pools, rois_t, grid_t):
    """From rois [R,4] compute gather indices and bilinear weights.

    Returns (idx_i32 [R, 98], wx_eff [R,7], wy_eff [R,7]).
    idx free layout: (jy, jx, yn) -> col jy*14 + jx*2 + yn. Each index is a row
    of the feats table; a gather of 512 elems covers pixel columns (bx, bx+1).
    """
    prep = pools["prep"]
    v = nc.vector

    def pt(cols, dtype=F32, tag=None):
        return prep.tile([R, cols], dtype, tag=tag, name=tag)

    y1 = rois_t[:, 0:1]
    x1 = rois_t[:, 1:2]
    y2 = rois_t[:, 2:3]
    x2 = rois_t[:, 3:4]

    hh = pt(1, tag="hh"); v.tensor_tensor(hh[:], y2, y1, op=Alu.subtract)
    ww = pt(1, tag="ww"); v.tensor_tensor(ww[:], x2, x1, op=Alu.subtract)
    hw = pt(1, tag="hw"); v.tensor_tensor(hw[:], hh[:], ww[:], op=Alu.mult)
    v.tensor_scalar(hw[:], hw[:], 1e-6, None, op0=Alu.max)

    # level selection: lvl = 2 + (hw>=112^2) + (hw>=224^2) + (hw>=448^2)
    g2 = pt(1, tag="g2"); v.tensor_scalar(g2[:], hw[:], 12544.0, None, op0=Alu.is_ge)
    g3 = pt(1, tag="g3"); v.tensor_scalar(g3[:], hw[:], 50176.0, None, op0=Alu.is_ge)
    g4 = pt(1, tag="g4"); v.tensor_scalar(g4[:], hw[:], 200704.0, None, op0=Alu.is_ge)

    # inv_stride = 0.25 - 0.125*g2 - 0.0625*g3 - 0.03125*g4  (exact)
    invs = pt(1, tag="invs")
    v.tensor_scalar(invs[:], g2[:], -0.125, 0.25, op0=Alu.mult, op1=Alu.add)
    t0 = pt(1, tag="t0")
    v.tensor_scalar(t0[:], g3[:], -0.0625, None, op0=Alu.mult)
    v.tensor_tensor(invs[:], invs[:], t0[:], op=Alu.add)
    v.tensor_scalar(t0[:], g4[:], -0.03125, None, op0=Alu.mult)
    v.tensor_tensor(invs[:], invs[:], t0[:], op=Alu.add)

    # feature side S = 1024 * inv_stride in {256,128,64,32}; level base offset
    S = pt(1, tag="S"); v.tensor_scalar(S[:], invs[:], 1024.0, None, op0=Alu.mult)
    base = pt(1, tag="base")
    v.tensor_scalar(base[:], g2[:], 65536.0, None, op0=Alu.mult)
    v.tensor_scalar(t0[:], g3[:], 16384.0, None, op0=Alu.mult)
    v.tensor_tensor(base[:], base[:], t0[:], op=Alu.add)
    v.tensor_scalar(t0[:], g4[:], 4096.0, None, op0=Alu.mult)
    v.tensor_tensor(base[:], base[:], t0[:], op=Alu.add)
    Sm1 = pt(1, tag="Sm1"); v.tensor_scalar(Sm1[:], S[:], -1.0, None, op0=Alu.add)
    Sm2 = pt(1, tag="Sm2"); v.tensor_scalar(Sm2[:], S[:], -2.0, None, op0=Alu.add)

    # scaled roi coords (exact: multiply by power of two)
    sy1 = pt(1, tag="sy1"); v.tensor_tensor(sy1[:], y1, invs[:], op=Alu.mult)
    sx1 = pt(1, tag="sx1"); v.tensor_tensor(sx1[:], x1, invs[:], op=Alu.mult)
    sy2 = pt(1, tag="sy2"); v.tensor_tensor(sy2[:], y2, invs[:], op=Alu.mult)
    sx2 = pt(1, tag="sx2"); v.tensor_tensor(sx2[:], x2, invs[:], op=Alu.mult)
    dy = pt(1, tag="dy"); v.tensor_tensor(dy[:], sy2[:], sy1[:], op=Alu.subtract)
    dx = pt(1, tag="dx"); v.tensor_tensor(dx[:], sx2[:], sx1[:], op=Alu.subtract)

    def axis_prep(scoord, dcoord, suffix, clamp):
        # ys = grid*d + s  (matches ref rounding: mult then add)
        ys = pt(POOL, tag="ys" + suffix)
        v.tensor_scalar(ys[:], grid_t[0:R, :], dcoord[:], scoord[:],
                        op0=Alu.mult, op1=Alu.add)
        # robust floor (works under truncation or round-to-nearest casts)
        yi = pt(POOL, I32, tag="yi" + suffix)
        v.tensor_copy(yi[:], ys[:])
        yf = pt(POOL, tag="yf" + suffix)
        v.tensor_copy(yf[:], yi[:])
        gt = pt(POOL, tag="gt" + suffix)
        v.tensor_tensor(gt[:], yf[:], ys[:], op=Alu.is_gt)
        y0f = pt(POOL, tag="y0f" + suffix)
        v.tensor_tensor(y0f[:], yf[:], gt[:], op=Alu.subtract)
        # clip to [0, S-1] (ys >= 0 so lower clip is a no-op)
        y0c = pt(POOL, tag="y0c" + suffix)
        v.tensor_scalar(y0c[:], y0f[:], Sm1[:], None, op0=Alu.min)
        # index clamp: x needs bx+1 <= S-1 (contiguous pair read); y is baked
        # into the paired table, so its row index is y0c itself.
        by = pt(POOL, tag="by" + suffix)
        v.tensor_scalar(by[:], y0c[:], clamp[:], None, op0=Alu.min)
        # weight: clip(ys - y0c, 0, 1); force 1.0 at the right edge
        wy = pt(POOL, tag="wy" + suffix)
        v.tensor_tensor(wy[:], ys[:], y0c[:], op=Alu.subtract)
        v.tensor_scalar(wy[:], wy[:], 0.0, 1.0, op0=Alu.max, op1=Alu.min)
        fl = pt(POOL, tag="fl" + suffix)
        v.tensor_scalar(fl[:], y0f[:], Sm1[:], None, op0=Alu.is_ge)
        v.tensor_tensor(wy[:], wy[:], fl[:], op=Alu.max)
        return by, wy

    by, wy_eff = axis_prep(sy1, dy, "Y", Sm1)
    bx, wx_eff = axis_prep(sx1, dx, "X", Sm2)

    # rowA = base + by*S  [R,7]
    rowA = pt(POOL, tag="rowA")
    v.tensor_scalar(rowA[:], by[:], S[:], base[:], op0=Alu.mult, op1=Alu.add)

    # idx layout [R, 7jy, 7jx]; one paired-table row index per bin. The
    # paired table row (y, x) holds [feat(y, x), feat(min(y+1, S-1), x)], so a
    # 1024-elem read from row (base + y*S + x) covers all 4 bilinear
    # neighbors: [f(y,x0), f(y+1,x0), f(y,x0+1), f(y+1,x0+1)].
    idxf = pools["idx"].tile([R, POOL * POOL], F32, tag="idxf", name="idxf")
    v3 = idxf[:].rearrange("p (a b) -> p a b", a=POOL)
    v.tensor_tensor(v3[:, :, :],
                    rowA[:, :, None].to_broadcast([R, POOL, POOL]),
                    bx[:, None, :].to_broadcast([R, POOL, POOL]),
                    op=Alu.add)
    idx_i32 = pools["idx"].tile([R, POOL * POOL], I32, tag="idxi", name="idxi")
    v.tensor_copy(idx_i32[:], idxf[:])
    return idx_i32, wx_eff, wy_eff


def _gather_interp_transpose(nc, pools, feats_ap, idx_i32, wx_eff, wy_eff,
                             ident_b, xt_tiles):
    """Fill xt_tiles (7 tiles of [128, 14*128]; K-block k at cols
    (k%14)*128, 125 used) with pooled features transposed (x^T for fc1).
    The psum->SBUF copy converts to each xt tile's dtype (bf16 or fp8e4)."""
    v = nc.vector
    # Per-stage x-interp diagonal weight tiles: Wd[jx][0] = diag(1-wx[:,jx]),
    # Wd[jx][1] = diag(wx[:,jx]). The x-interp then folds into the PE
    # "transposes" as psum-accumulated diagonal matmuls:
    #   xt[c, roi] = (1-wx)(roi)*T0[roi, c] + wx(roi)*T1[roi, c]
    wxm = pools["prep"].tile([R, POOL], F32, tag="wxm", name="wxm")
    v.tensor_scalar(wxm[:], wx_eff[:], -1.0, 1.0, op0=Alu.mult, op1=Alu.add)
    wxd = pools["wxd"].tile([R, POOL * 2 * 128], BF16, tag="wxd", name="wxd")
    wdv = wxd[:].rearrange("p (b x q) -> p b x q", b=POOL, x=2)
    for jx in range(POOL):
        v.tensor_scalar(wdv[:, jx, 0, :], ident_b[0:R, 0:128],
                        wxm[:, jx:jx + 1], None, op0=Alu.mult)
        v.tensor_scalar(wdv[:, jx, 1, :], ident_b[0:R, 0:128],
                        wx_eff[:, jx:jx + 1], None, op0=Alu.mult)
    for jy in range(POOL):
        G = pools["gath"].tile([R, 7 * 1024], FP8, tag="G", name="G")
        # HW indirect DMA consumes exactly one index per output partition,
        # so issue one gather per jx bin; the paired table packs all 4
        # bilinear neighbors into one 1024-elem contiguous read.
        for j in range(POOL):
            nc.gpsimd.indirect_dma_start(
                out=G[:, j * 1024:(j + 1) * 1024],
                out_offset=None,
                in_=feats_ap[:],
                in_offset=bass.IndirectOffsetOnAxis(
                    ap=idx_i32[:, jy * POOL + j:jy * POOL + j + 1], axis=0),
            )
        # layout [R, jx, xn, yn, c]; interp math lands in a bf16 tile so
        # no intermediate is ever quantized to fp8
        gv = G[:].rearrange("p (b x y e) -> p b x y e", b=POOL, x=2, y=2)
        T = pools["yint"].tile([R, POOL * 2 * 256], BF16, tag="T", name="T")
        tv = T[:].rearrange("p (b x e) -> p b x e", b=POOL, x=2)
        # y-interp: T = G0 + wy*(G1 - G0)
        v.tensor_tensor(tv, gv[:, :, :, 1, :], gv[:, :, :, 0, :],
                        op=Alu.subtract)
        nc.scalar.mul(tv, tv, wy_eff[:, jy:jy + 1])
        v.tensor_tensor(tv, tv, gv[:, :, :, 0, :], op=Alu.add)
        # x-interp + transpose fused on the PE: per (jx, c-half) accumulate
        # the two xn diagonal matmuls into one psum slot; two slots share a
        # PSUM bank so each psum->SBUF copy moves a pair.
        xt = xt_tiles[jy]
        xv = xt[:].rearrange("p (t c) -> p t c", c=128)
        for jx in range(POOL):
            ps = pools["pt"].tile([128, 2, 128], F32, space="PSUM",
                                  tag="ptrf", name="ptrf")
            for u in range(2):
                for xn in range(2):
                    nc.tensor.matmul(
                        ps[:, u, :],
                        lhsT=tv[:, jx, xn, u * 128:(u + 1) * 128],
                        rhs=wdv[:, jx, xn, :],
                        start=(xn == 0), stop=(xn == 1))
            nc.scalar.copy(xv[:, 2 * jx:2 * jx + 2, :], ps[:])


def _fc(nc, pools, xt_slices, KT, w_tiled, b2, head, n_out, relu, ones, w_tag,
        wdt, chunk, inv_scale=1.0, h_dt=F32):
    """h[R, n_out] = act((x @ W)*inv_scale + b*inv_scale).

    W pre-tiled on host as [3, KT//chunk, 128, chunk*n_out]; b pre-scaled by
    1/inv_scale so the single post-GEMM activation descale is exact.
    xt_slices(k) -> lhsT AP [128, R]. PSUM matmul outputs must stay within
    one 2KB bank, hence the 512-column psum split."""
    offs = list(range(0, n_out, 512))
    sizes = [min(512, n_out - o) for o in offs]
    psums = [pools["pfc"].tile([128, 512], F32, space="PSUM", tag=f"ps{j}",
                               name=f"ps{j}")
             for j in range(len(offs))]
    dma_eng = nc.sync if w_tag == "w1" else nc.scalar
    n_ch = KT // chunk
    for c in range(n_ch):
        wt = pools[w_tag].tile([128, chunk * n_out], wdt, tag=w_tag, name=w_tag)
        dma_eng.dma_start(wt[:], w_tiled[head, c])
        for j in range(chunk):
            k = c * chunk + j
            for jj, (o, sz) in enumerate(zip(offs, sizes)):
                nc.tensor.matmul(psums[jj][:, 0:sz], lhsT=xt_slices(k),
                                 rhs=wt[:, j * n_out + o:j * n_out + o + sz],
                                 start=(k == 0), stop=False)
    bt = pools["bias"].tile([1, n_out], F32, tag="bias", name="bias")
    nc.scalar.dma_start(bt[:], b2[head][None, :])
    for jj, (o, sz) in enumerate(zip(offs, sizes)):
        nc.tensor.matmul(psums[jj][:, 0:sz], lhsT=ones[0:1, 0:128],
                         rhs=bt[0:1, o:o + sz], start=False, stop=True)
    h = pools["h"].tile([128, n_out], h_dt, tag=f"h{n_out}", name=f"h{n_out}")
    for jj, (o, sz) in enumerate(zip(offs, sizes)):
        if relu:
            nc.scalar.activation(h[:, o:o + sz], psums[jj][:, 0:sz], Act.Relu,
                                 scale=inv_scale)
        elif inv_scale != 1.0:
            nc.scalar.activation(h[:, o:o + sz], psums[jj][:, 0:sz], Act.Copy,
                                 scale=inv_scale)
        else:
            nc.scalar.copy(h[:, o:o + sz], psums[jj][:, 0:sz])
    return h


def _transpose_h(nc, pools, h, ident):
    """h [128, 1024] fp32 (padding rows finite) -> hT [128, 1024] bf16."""
    hT = pools["ht"].tile([128, HID], BF16, tag="hT", name="hT")
    hv = hT[:].rearrange("p (t c) -> p t c", c=128)
    for t in range(0, HID // 128, 2):
        ps = pools["pt"].tile([128, 2, 128], F32, space="PSUM", tag="ptrf",
                              name="ptrf")
        for u in range(2):
            nc.tensor.transpose(out=ps[:, u, :],
                                in_=h[:, (t + u) * 128:(t + u + 1) * 128],
                                identity=ident[:])
        nc.scalar.copy(hv[:, t:t + 2, :], ps[:])
    return hT


def _softmax(nc, pools, logits_t):
    v = nc.vector
    logits = logits_t[0:R, :]
    rmax = pools["prep"].tile([R, 1], F32, tag="rmax", name="rmax")
    v.tensor_reduce(rmax[:], logits, axis=mybir.AxisListType.X, op=Alu.max)
    nmax = pools["prep"].tile([R, 1], F32, tag="nmax", name="nmax")
    v.tensor_scalar(nmax[:], rmax[:], -1.0, None, op0=Alu.mult)
    e = pools["h"].tile([R, NCLS], F32, tag="smx", name="smx")
    nc.scalar.activation(e[:], logits, Act.Exp, bias=nmax[:], scale=1.0)
    ssum = pools["prep"].tile([R, 1], F32, tag="ssum", name="ssum")
    v.tensor_reduce(ssum[:], e[:], axis=mybir.AxisListType.X, op=Alu.add)
    rsum = pools["prep"].tile([R, 1], F32, tag="rsum", name="rsum")
    v.reciprocal(rsum[:], ssum[:])
    v.tensor_scalar(e[:], e[:], rsum[:], None, op0=Alu.mult)
    return e


def _delta2bbox(nc, pools, rois_t, deltas, stds_t, rois_pool):
    """rois_next = delta2bbox(rois_t, deltas) following the reference op order."""
    v = nc.vector
    prep = pools["prep"]

    def pt(tag):
        return prep.tile([R, 1], F32, tag=tag, name=tag)

    d = prep.tile([R, 4], F32, tag="dsc", name="dsc")
    v.tensor_tensor(d[:], deltas[0:R, :], stds_t[0:R, :], op=Alu.mult)
    y1 = rois_t[:, 0:1]; x1 = rois_t[:, 1:2]; y2 = rois_t[:, 2:3]; x2 = rois_t[:, 3:4]
    hh = pt("b_h"); v.tensor_tensor(hh[:], y2, y1, op=Alu.subtract)
    ww = pt("b_w"); v.tensor_tensor(ww[:], x2, x1, op=Alu.subtract)
    # cy = (y1 + 0.5*h) + d0*h ; cx likewise
    hh2 = pt("b_h2"); v.tensor_scalar(hh2[:], hh[:], 0.5, None, op0=Alu.mult)
    cy = pt("b_cy"); v.tensor_tensor(cy[:], y1, hh2[:], op=Alu.add)
    t = pt("b_t"); v.tensor_tensor(t[:], d[:, 0:1], hh[:], op=Alu.mult)
    v.tensor_tensor(cy[:], cy[:], t[:], op=Alu.add)
    ww2 = pt("b_w2"); v.tensor_scalar(ww2[:], ww[:], 0.5, None, op0=Alu.mult)
    cx = pt("b_cx"); v.tensor_tensor(cx[:], x1, ww2[:], op=Alu.add)
    v.tensor_tensor(t[:], d[:, 1:2], ww[:], op=Alu.mult)
    v.tensor_tensor(cx[:], cx[:], t[:], op=Alu.add)
    # h' = h*exp(d2), w' = w*exp(d3)
    eh = pt("b_eh"); nc.scalar.activation(eh[:], d[:, 2:3], Act.Exp)
    ew = pt("b_ew"); nc.scalar.activation(ew[:], d[:, 3:4], Act.Exp)
    v.tensor_tensor(hh[:], hh[:], eh[:], op=Alu.mult)
    v.tensor_tensor(ww[:], ww[:], ew[:], op=Alu.mult)
    v.tensor_scalar(hh2[:], hh[:], 0.5, None, op0=Alu.mult)
    v.tensor_scalar(ww2[:], ww[:], 0.5, None, op0=Alu.mult)
    rn = rois_pool.tile([R, 4], F32, tag="rois", name="rois")
    v.tensor_tensor(rn[:, 0:1], cy[:], hh2[:], op=Alu.subtract)
    v.tensor_tensor(rn[:, 1:2], cx[:], ww2[:], op=Alu.subtract)
    v.tensor_tensor(rn[:, 2:3], cy[:], hh2[:], op=Alu.add)
    v.tensor_tensor(rn[:, 3:4], cx[:], ww2[:], op=Alu.add)
    for j in range(4):
        v.tensor_scalar(rn[:, j:j + 1], rn[:, j:j + 1], 0.0, IMG,
                        op0=Alu.max, op1=Alu.min)
    return rn


def build_kernel(ctx: ExitStack, tc: "tile.TileContext", aps: dict):
    nc = tc.nc
    pools = {}
    for name, bufs, space in [
        ("const", 1, "SBUF"), ("rois", 2, "SBUF"), ("prep", 2, "SBUF"),
        ("idx", 2, "SBUF"), ("gath", 4, "SBUF"), ("yint", 4, "SBUF"),
        ("wxd", 2, "SBUF"),
        ("xt", 1, "SBUF"), ("w1", 4, "SBUF"), ("w2", 2, "SBUF"),
        ("wsm", 2, "SBUF"), ("bias", 2, "SBUF"), ("h", 2, "SBUF"),
        ("ht", 2, "SBUF"), ("acc", 1, "SBUF"),
        ("pt", 3, "PSUM"), ("pfc", 2, "PSUM"),
    ]:
        pools[name] = ctx.enter_context(tc.tile_pool(name=name, bufs=bufs,
                                                     space=space))

    ident = pools["const"].tile([128, 128], F32, tag="ident", name="ident")
    make_identity(nc, ident[:])
    ident_b = pools["const"].tile([128, 128], BF16, tag="identb", name="identb")
    nc.vector.tensor_copy(ident_b[:], ident[:])
    ones = pools["const"].tile([1, 128], F32, tag="ones", name="ones")
    nc.vector.memset(ones[:], 1.0)
    grid_t = pools["const"].tile([128, POOL], F32, tag="grid", name="grid")
    nc.sync.dma_start(grid_t[:], aps["grid_c"][:])
    stds_t = pools["const"].tile([128, 4], F32, tag="stds", name="stds")
    nc.sync.dma_start(stds_t[:], aps["stds_c"][:])

    rois_t = pools["rois"].tile([R, 4], F32, tag="rois", name="rois")
    nc.sync.dma_start(rois_t[:], aps["rois"][:])

    def head_app(i, xt_tiles, want):
        def xt_slice(k):
            return xt_tiles[k // 14][:, (k % 14) * 128:(k % 14 + 1) * 128]

        h1 = _fc(nc, pools, xt_slice, KT1, aps["fc1_w8"], aps["fc1_b"],
                 i, HID, True, ones, "w1", FP8, CH1, inv_scale=1.0 / S1)
        h1T = _transpose_h(nc, pools, h1, ident)

        def h1t_slice(k):
            return h1T[:, k * 128:(k + 1) * 128]

        h2 = _fc(nc, pools, h1t_slice, KT2, aps["fc2_w8"], aps["fc2_b"],
                 i, HID, True, ones, "w2", FP8, KT2, inv_scale=1.0 / S2)
        h2T = _transpose_h(nc, pools, h2, ident)

        def h2t_slice(k):
            return h2T[:, k * 128:(k + 1) * 128]

        if want == "deltas":
            return _fc(nc, pools, h2t_slice, KT2, aps["reg_wb"],
                       aps["reg_b"], i, 4, False, ones, "wsm", BF16, KT2)
        logits = _fc(nc, pools, h2t_slice, KT2, aps["cls_wb"],
                     aps["cls_b"], i, NCLS, False, ones, "wsm", BF16, KT2)
        return _softmax(nc, pools, logits)

    acc = pools["acc"].tile([R, NCLS], F32, tag="acc", name="acc")
    xt_tiles = None
    for s in range(N_STAGES):
        idx_i32, wx_eff, wy_eff = _roi_prep(nc, pools, rois_t, grid_t)
        xt_tiles = [pools["xt"].tile([128, 14 * 128], BF16, tag=f"xt{j}",
                                     name=f"xt{j}")
                    for j in range(POOL)]
        _gather_interp_transpose(nc, pools, aps["feats_b"], idx_i32, wx_eff,
                                 wy_eff, ident_b, xt_tiles)
        if s < 2:
            deltas = head_app(s, xt_tiles, "deltas")
            rois_t = _delta2bbox(nc, pools, rois_t, deltas, stds_t,
                                 pools["rois"])
        else:
            p3 = head_app(2, xt_tiles, "probs")
            nc.vector.tensor_copy(acc[:], p3[:])

    for i in range(2):
        pi = head_app(i, xt_tiles, "probs")
        nc.vector.tensor_tensor(acc[:], acc[:], pi[:], op=Alu.add)

    outp = pools["h"].tile([R, NCLS], F32, tag="outp", name="outp")
    nc.vector.tensor_scalar(outp[:], acc[:], 1.0 / 3.0, None, op0=Alu.mult)
    nc.sync.dma_start(aps["out"][:], outp[:])


# ---------------------------------------------------------------------------
# host side
# ---------------------------------------------------------------------------

_CACHE: dict = {}


def build_program(reps: int = 1):
    nc = bacc.Bacc("TRN2", target_bir_lowering=False, debug=False,
                   num_devices=N_CORES)
    aps = {
        "feats_b": nc.dram_tensor("feats_b", [FEAT_ROWS, 2 * C], FP8,
                                  kind="ExternalInput").ap(),
        "rois": nc.dram_tensor("rois", [R, 4], F32, kind="ExternalInput").ap(),
        "fc1_w8": nc.dram_tensor("fc1_w8", [3, KT1 // CH1, 128, CH1 * HID],
                                 FP8, kind="ExternalInput").ap(),
        "fc1_b": nc.dram_tensor("fc1_b", [3, HID], F32,
                                kind="ExternalInput").ap(),
        "fc2_w8": nc.dram_tensor("fc2_w8", [3, 1, 128, KT2 * HID], FP8,
                                 kind="ExternalInput").ap(),
        "fc2_b": nc.dram_tensor("fc2_b", [3, HID], F32,
                                kind="ExternalInput").ap(),
        "cls_wb": nc.dram_tensor("cls_wb", [3, 1, 128, KT2 * NCLS], BF16,
                                 kind="ExternalInput").ap(),
        "cls_b": nc.dram_tensor("cls_b", [3, NCLS], F32,
                                kind="ExternalInput").ap(),
        "reg_wb": nc.dram_tensor("reg_wb", [3, 1, 128, KT2 * 4], BF16,
                                 kind="ExternalInput").ap(),
        "reg_b": nc.dram_tensor("reg_b", [3, 4], F32,
                                kind="ExternalInput").ap(),
        "grid_c": nc.dram_tensor("grid_c", [128, POOL], F32,
                                 kind="ExternalInput").ap(),
        "stds_c": nc.dram_tensor("stds_c", [128, 4], F32,
                                 kind="ExternalInput").ap(),
        "out": nc.dram_tensor("out", [R, NCLS], F32,
                              kind="ExternalOutput").ap(),
    }
    with tile.TileContext(nc) as tc:
        for _ in range(reps):
            with ExitStack() as ctx:
                build_kernel(ctx, tc, aps)
    nc.compile()
    return nc


def host_consts():
    grid = ((np.arange(POOL, dtype=np.float32) + np.float32(0.5))
            / np.float32(POOL))
    grid_c = np.broadcast_to(grid, (128, POOL)).copy()
    stds_c = np.broadcast_to(
        np.array([0.1, 0.1, 0.2, 0.2], dtype=np.float32), (128, 4)).copy()
    return grid_c, stds_c


def _tile_w(w, kt, chunk, n_out):
    """[3, K, n_out] -> [3, K//(128*chunk), 128, chunk*n_out] k-tiled layout:
    row p of tile (c, j) holds w[c*chunk*128 + j*128 + p]."""
    w = np.ascontiguousarray(w)
    n_ch = kt // chunk
    wt = (w.reshape(3, n_ch, chunk, 128, n_out)
          .transpose(0, 1, 3, 2, 4)
          .reshape(3, n_ch, 128, chunk * n_out))
    return np.ascontiguousarray(wt)


def make_in_maps(inputs: dict) -> list:
    import ml_dtypes
    f32 = lambda x: np.ascontiguousarray(np.asarray(x, dtype=np.float32))
    bf16 = lambda x: np.ascontiguousarray(
        np.asarray(x, dtype=np.float32).astype(ml_dtypes.bfloat16))

    def fp8(x, scale):
        y = np.clip(np.asarray(x, np.float32) * scale, -15.0, 15.0)
        return np.ascontiguousarray(y.astype(ml_dtypes.float8_e3m4))

    def pair_level(p):
        # [S, S, C] -> [S*S, 2C]: row (y,x) = [feat(y,x), feat(min(y+1,S-1),x)]
        f = f32(p)[0]
        fn = np.concatenate([f[1:], f[-1:]], axis=0)
        return np.concatenate([f, fn], axis=2).reshape(-1, 2 * C)

    feats = np.concatenate([pair_level(inputs[k])
                            for k in ("P2", "P3", "P4", "P5")], axis=0)
    feats = np.ascontiguousarray(
        np.clip(feats, -15.0, 15.0).astype(ml_dtypes.float8_e3m4))
    grid_c, stds_c = host_consts()
    rois = f32(inputs["rois"])
    shared = {
        "feats_b": feats,
        "fc1_w8": _tile_w(fp8(inputs["fc1_w"], S1), KT1, CH1, HID),
        "fc1_b": f32(inputs["fc1_b"]) * np.float32(S1),
        "fc2_w8": _tile_w(fp8(inputs["fc2_w"], S2), KT2, KT2, HID),
        "fc2_b": f32(inputs["fc2_b"]) * np.float32(S2),
        "cls_wb": _tile_w(bf16(inputs["cls_w"]), KT2, KT2, NCLS),
        "cls_b": f32(inputs["cls_b"]),
        "reg_wb": _tile_w(bf16(inputs["reg_w"]), KT2, KT2, 4),
        "reg_b": f32(inputs["reg_b"]),
        "grid_c": grid_c, "stds_c": stds_c,
    }
    return [dict(shared, rois=rois[c * R:(c + 1) * R]) for c in range(N_CORES)]


def make_runner(nc):
    """Jitted SPMD executor: rois/outputs sharded over cores, all other
    inputs replicated (avoids the 8x host-side concat of the big weights)."""
    import jax
    from jax.sharding import Mesh, PartitionSpec
    from jax.experimental.shard_map import shard_map
    from concourse import bass2jax

    bass2jax.install_neuronx_cc_hook()
    pname = nc.partition_id_tensor.name if nc.partition_id_tensor else None
    in_names, out_names, out_avals = [], [], []
    for alloc in nc.m.functions[0].allocations:
        if not isinstance(alloc, mybir.MemoryLocationSet):
            continue
        name = alloc.memorylocations[0].name
        if alloc.kind == "ExternalInput":
            if name != pname:
                in_names.append(name)
        elif alloc.kind == "ExternalOutput":
            out_names.append(name)
            out_avals.append(jax.core.ShapedArray(
                tuple(alloc.tensor_shape), mybir.dt.np(alloc.dtype)))
    n_outs = len(out_avals)
    names_full = list(in_names) + out_names + ([pname] if pname else [])

    def _body(*args):
        ops = list(args)
        if pname is not None:
            ops.append(bass2jax.partition_id_tensor())
        return tuple(bass2jax._bass_exec_p.bind(
            *ops, out_avals=tuple(out_avals), in_names=tuple(names_full),
            out_names=tuple(out_names), lowering_input_output_aliases=(),
            sim_require_finite=True, sim_require_nnan=True, nc=nc))

    devices = jax.devices()[:N_CORES]
    mesh = Mesh(np.asarray(devices), ("core",))
    P_ = PartitionSpec
    in_specs = tuple(P_("core") if nm == "rois" else P_() for nm in in_names) \
        + (P_("core"),) * n_outs
    sharded = jax.jit(
        shard_map(_body, mesh=mesh, in_specs=in_specs,
                  out_specs=(P_("core"),) * n_outs, check_rep=False),
        keep_unused=True)

    def _args(shared, rois_full):
        args = [rois_full if nm == "rois" else shared[nm] for nm in in_names]
        args += [np.zeros((N_CORES * a.shape[0], *a.shape[1:]), a.dtype)
                 for a in out_avals]
        return args

    def prepare(shared: dict, rois_full: np.ndarray):
        from jax.sharding import NamedSharding
        args = _args(shared, rois_full)
        shards = [NamedSharding(mesh, s) for s in in_specs]
        return [jax.device_put(a, s) for a, s in zip(args, shards)]

    def run_dev(dev_args):
        out = sharded(*dev_args)
        jax.block_until_ready(out)
        return np.asarray(out[0])

    def run(shared: dict, rois_full: np.ndarray):
        out = sharded(*_args(shared, rois_full))
        jax.block_until_ready(out)
        return np.asarray(out[0])

    run.prepare = prepare
    run.run_dev = run_dev
    run.sharded = sharded
    return run


def kernel(**inputs) -> np.ndarray:
    if "nc" not in _CACHE:
        _CACHE["nc"] = build_program()
        _CACHE["run"] = make_runner(_CACHE["nc"])
    in_maps = make_in_maps(inputs)
    shared = dict(in_maps[0])
    rois_full = np.ascontiguousarray(np.asarray(inputs["rois"], np.float32))
    out = _CACHE["run"](shared, rois_full)
    return out.astype(np.float32)
